# revision 9
# baseline (speedup 1.0000x reference)
"""3-layer GCN (PyG GCNConv x3, relu between) on 8 Trainium2 NeuronCores.

Math: out = A*(relu(A*(relu(A*(xW1)+b1)W2+b2))W3)+b3 with A = D^-1/2(A+I)D^-1/2.
Per layer: htilde = dinv * (input @ W) (dense, PE, bf16) is written to a
per-core slab and AllGathered (in 4 chunks, overlapped with the producer
loop) into a shared 100,416-row table; aggregation of the REAL edges is a
dma_gather + per-block strided tensor_reduce; the self-loop term
dinv^2*(input@W)[dst] + b is kept in SBUF from the transform phase and added
via one scalar_tensor_tensor, so self-loops never transit the gather path.

Gather indexing: dma_gather takes int16 row indices. The table is split in 2
classes (= 2 halves of 50,208 rows); each call's in_ap is based at the middle
of its half, idx = row - mid in [-25104, 25103] (HW sign-extends; only a
negative index in the *final* list position is dropped, so pads are positive
and each call's last entry is forced non-negative by an in-lane swap).
Host preprocessing greedily 2-colors SRC nodes to split every dst's in-edges
evenly across the halves, then packs dsts with similar (deg, class-0 count)
into the same 128-lane block (snake order) so per-block per-class k-extents
hug the actual counts. Consecutive blocks of one class share a single gather
call (up to 4096 idxs) to amortize the ~1us SWDGE fixed cost.

Layout: dst slot (core, block j, lane p). xT columns are block-major
(j*128+p); table/slab rows are lane-major within each chunk
(p*chunk_blocks + j - B0) so 8-block batches of transform output form one
contiguous-per-partition DMA; out rows are lane-major global (p*98+j).
Each (core, chunk) appends 2 zero slots (gather pad target, upper half).
"""
import sys
sys.path.insert(0, "/opt/trn_rl_repo")
import numpy as np

N = 100_000
DIMS = [512, 128, 64, 32]
NCORES = 8
P = 128
NBLK = 98
GD = 128
SLOTS = NBLK * P                 # 12544 xT columns / out rows per core
NQ = 2
CLSBLK = NBLK // NQ              # 49 blocks per class
CROWS = CLSBLK * P + 2           # per-core rows per class (incl 2 zero slots)
SLABROWS = NQ * CROWS            # 12548
CLSROWS = NCORES * CROWS         # 50192 rows per class table
MID = CLSROWS // 2               # 25096
PADIDX = MID - 1                 # core-7 zero row (upper half)
CALL_CAP_COLS = 32               # 4096 idxs per gather call
WB = 8                           # transform write batch (blocks)

_CACHE = {}


# --------------------------------------------------------------------------
# host-side graph preprocessing
# --------------------------------------------------------------------------
def _preprocess(edge_index):
    src = np.asarray(edge_index[0], np.int64)
    dst = np.asarray(edge_index[1], np.int64)
    indeg = np.bincount(dst, minlength=N).astype(np.int64)
    deg = indeg + 1                                   # + self loop (norm)
    dinv = (1.0 / np.sqrt(deg)).astype(np.float32)

    # ---- 2-coloring of SRC nodes: balance each dst's in-edges across halves
    order = np.argsort(src, kind="stable")
    S_srt, D_srt = src[order], dst[order]
    indptr = np.searchsorted(S_srt, np.arange(N + 1))
    M = np.zeros((N, NQ), np.int32)
    cls = np.full(N, -1, np.int8)
    capq = np.full(NQ, 49 * 1024, np.int64)
    proc = np.argsort(-(indptr[1:] - indptr[:-1]), kind="stable")
    for n in proc:
        nb = D_srt[indptr[n]:indptr[n + 1]]
        diff = int((M[nb, 0] - M[nb, 1]).sum())
        q = 0 if diff < 0 else 1
        if capq[q] <= 0:
            q = 1 - q
        cls[n] = q
        capq[q] -= 1
        M[nb, q] += 1
    for _ in range(6):
        moved = 0
        for n in proc:
            nb = D_srt[indptr[n]:indptr[n + 1]]
            q0 = cls[n]
            M[nb, q0] -= 1
            capq[q0] += 1
            diff = int((M[nb, 0] - M[nb, 1]).sum())
            q = 0 if diff < 0 else 1
            if capq[q] <= 0:
                q = 1 - q
            moved += q != q0
            cls[n] = q
            capq[q] -= 1
            M[nb, q] += 1
        if moved == 0:
            break

    # ---- pack dsts into blocks (snake order on (deg, c0)); class q -> its
    # 49 blocks; within a block, position t -> core t//128, lane t%128
    core_of = np.full(N, -1, np.int64)
    blk_of = np.full(N, -1, np.int64)
    lane_of = np.full(N, -1, np.int64)
    for q in range(NQ):
        nodes_q = np.flatnonzero(cls == q)
        c0 = M[nodes_q, 0].astype(np.int64)
        d = c0 + M[nodes_q, 1]
        key2 = np.where(d % 2 == 0, c0, 100000 - c0)
        stream = nodes_q[np.lexsort((key2, d))]
        jbase = q * CLSBLK
        for b in range(CLSBLK):
            seg = stream[b * 1024:(b + 1) * 1024]
            t = np.arange(len(seg))
            core_of[seg] = t // P
            blk_of[seg] = jbase + b
            lane_of[seg] = t % P

    # ---- table rows: class tensor row = c*CROWS + p*CLSBLK + (j - q*CLSBLK)
    ec, ej, el = core_of[dst], blk_of[dst], lane_of[dst]
    eq = cls[src].astype(np.int64)
    srow = (core_of[src] * CROWS + lane_of[src] * CLSBLK
            + (blk_of[src] - eq * CLSBLK))
    eidx = srow - MID
    assert eidx.min() >= -MID and eidx.max() < MID

    # per-(core, class, block, lane) counts -> K[q, j]
    key = ((ec * NQ + eq) * NBLK + ej) * P + el
    cnt = np.zeros(NCORES * NQ * NBLK * P, np.int64)
    np.add.at(cnt, key, 1)
    cnt4 = cnt.reshape(NCORES, NQ, NBLK, P)
    K = cnt4.max(axis=(0, 3))                         # [NQ, NBLK]

    order_e = np.argsort(key, kind="stable")
    ks = key[order_e]
    first = np.flatnonzero(np.r_[True, ks[1:] != ks[:-1]])
    within = np.arange(len(ks)) - first[np.searchsorted(ks[first], ks)]
    oc, oq, oj, ol = ec[order_e], eq[order_e], ej[order_e], el[order_e]
    oidx = eidx[order_e]

    PADIDX = MID - 1                                  # core-7 zero row, upper

    def build(K):
        # calls: per class, consecutive blocks, <= CALL_CAP_COLS columns
        calls = []                                    # (q, [(j, off, K)], cols)
        for q in range(NQ):
            cur, cols = [], 0
            for j in range(NBLK):
                kqj = int(K[q, j])
                if kqj == 0:
                    continue
                if cols + kqj > CALL_CAP_COLS and cur:
                    calls.append((q, cur, cols))
                    cur, cols = [], 0
                cur.append((j, cols, kqj))
                cols += kqj
            if cur:
                calls.append((q, cur, cols))
        # class-1 calls are staggered ~10 blocks later in issue order: the
        # gpsimd queue is in-order, and the first class-1 call blocks on the
        # class-1 AllGather mesh; meanwhile class-0 calls (whose AG completed
        # mid-producer-loop) keep the DMA engines fed.
        calls.sort(key=lambda cc: (cc[1][0][0] + (10 if cc[0] == 1 else 0), cc[0]))
        cbase = np.zeros(len(calls), np.int64)
        acc = 0
        cidm = np.full((NQ, NBLK), -1, np.int64)
        offm = np.zeros((NQ, NBLK), np.int64)
        blkq = [[None] * NQ for _ in range(NBLK)]
        for cid, (q, blks, cols) in enumerate(calls):
            cbase[cid] = acc
            acc += cols * P
            for (j, off, kqj) in blks:
                cidm[q, j], offm[q, j] = cid, off
                blkq[j][q] = (cid, off, kqj)
        total_idx = acc

        req = np.full((NCORES, total_idx), PADIDX, np.int64)
        ecall = cidm[oq, oj]
        ekoff = offm[oq, oj] + within
        req[oc, cbase[ecall] + ekoff * P + ol] = oidx
        return calls, cbase, blkq, total_idx, req

    for _ in range(10):
        calls, cbase, blkq, total_idx, req = build(K)
        bumped = False
        for cid, (q, blks, cols) in enumerate(calls):
            e = cbase[cid] + cols * P - 1
            jl, offl, kl = blks[-1]
            # final entry = lane 127, top-k of the call's last block; if
            # negative, swap with a non-negative entry in the same lane/block
            # k-range (order within a k-range is irrelevant to the sum)
            cols_l = cbase[cid] + (offl + np.arange(kl)) * P + (P - 1)
            for c in range(NCORES):
                if req[c, e] < 0:
                    cand = np.flatnonzero(req[c, cols_l] >= 0)
                    if len(cand) == 0:
                        K[q, jl] += 1
                        bumped = True
                        break
                    s_ = cols_l[cand[0]]
                    req[c, e], req[c, s_] = req[c, s_], req[c, e]
            if bumped:
                break
        if not bumped:
            break
    assert req.max() < 32768 and req.min() >= -32768

    wrapped = np.empty((NCORES, P, total_idx // 16), np.int16)
    for c in range(NCORES):
        col = 0
        for cid, (q, blks, cols) in enumerate(calls):
            L = req[c, cbase[cid]:cbase[cid] + cols * P]
            w = L.reshape(-1, 16).T.astype(np.int16)
            wrapped[c, :, col:col + cols * P // 16] = np.tile(w, (8, 1))
            col += cols * P // 16

    # ---- per-core node placement tables
    ids = np.full((NCORES, NBLK, P), -1, np.int64)    # [c, j, p] -> node
    m = core_of >= 0
    ids[core_of[m], blk_of[m], lane_of[m]] = np.flatnonzero(m)
    dinv_t = np.zeros((NCORES, P, NBLK), np.float32)
    dinv2_t = np.zeros((NCORES, P, NBLK), np.float32)
    for c in range(NCORES):
        v = ids[c]                                    # [j, p]
        dv = np.where(v >= 0, dinv[np.maximum(v, 0)], 0.0).astype(np.float32)
        dinv_t[c] = dv.T
        dinv2_t[c] = (dv * dv).T
    return dict(ids=ids, calls=calls, cbase=cbase, blkq=blkq,
                wrapped=wrapped, dinv_t=dinv_t, dinv2_t=dinv2_t,
                total_idx=total_idx)


# --------------------------------------------------------------------------
# bass program
# --------------------------------------------------------------------------
def _build(pre):
    from concourse import bass, bacc, mybir, tile
    from concourse.library_config import mlp
    from concourse.masks import make_identity
    AL = mybir.AluOpType
    f32, bf16, i16 = mybir.dt.float32, mybir.dt.bfloat16, mybir.dt.int16
    calls, blkq, cbase = pre["calls"], pre["blkq"], pre["cbase"]
    total_idx = pre["total_idx"]

    nc = bacc.Bacc("TRN2", target_bir_lowering=False, debug=False,
                   num_devices=NCORES, num_swdge_queues=4)
    xT_in = nc.dram_tensor("xT", (DIMS[0], SLOTS), bf16, kind="ExternalInput")
    idx_in = nc.dram_tensor("gidx", (P, total_idx // 16), i16, kind="ExternalInput")
    dinv_in = nc.dram_tensor("dinv", (P, NBLK), f32, kind="ExternalInput")
    dinv2_in = nc.dram_tensor("dinv2", (P, NBLK), f32, kind="ExternalInput")
    w_in = [nc.dram_tensor("W1", (DIMS[0], P), bf16, kind="ExternalInput"),
            nc.dram_tensor("W2", (P, P), bf16, kind="ExternalInput"),
            nc.dram_tensor("W3", (DIMS[2], P), bf16, kind="ExternalInput")]
    b_in = [nc.dram_tensor(f"b{i+1}", (P, DIMS[i + 1]), f32, kind="ExternalInput")
            for i in range(3)]
    out_t = nc.dram_tensor("out", (SLOTS, DIMS[3]), f32, kind="ExternalOutput")

    with tile.TileContext(nc) as tc:
        with tc.tile_pool(name="const", bufs=1) as const, \
             tc.tile_pool(name="selfp", bufs=1) as selfp, \
             tc.tile_pool(name="gath", bufs=10) as gpool, \
             tc.tile_pool(name="work", bufs=4) as work, \
             tc.tile_pool(name="hbt", bufs=3) as hbt, \
             tc.tile_pool(name="lhs", bufs=2) as lhs, \
             tc.tile_pool(name="pps", bufs=2, space="PSUM") as pps, \
             tc.tile_pool(name="ppt", bufs=2, space="PSUM") as ppt, \
             tc.tile_pool(name="dram", bufs=1, space="DRAM") as dram:

            nc.gpsimd.load_library(mlp)
            idx_t = const.tile([P, total_idx // 16], i16, tag="idx")
            nc.sync.dma_start(out=idx_t[:], in_=idx_in[:, :])
            dinv_t = const.tile([P, NBLK], f32, tag="dinv")
            nc.sync.dma_start(out=dinv_t[:], in_=dinv_in[:, :])
            dinv2_t = const.tile([P, NBLK], f32, tag="dinv2")
            nc.sync.dma_start(out=dinv2_t[:], in_=dinv2_in[:, :])
            ident = const.tile([P, P], f32, tag="ident")
            make_identity(nc, ident[:])
            zt = const.tile([P, GD], bf16, tag="zero")
            nc.vector.memset(zt[:], 0.0)
            w1t = [const.tile([P, P], bf16, tag=f"w1_{k}", name=f"w1_{k}") for k in range(4)]
            for k in range(4):
                nc.sync.dma_start(out=w1t[k][:], in_=w_in[0][k * P:(k + 1) * P, :])
            w2t = const.tile([P, P], bf16, tag="w2")
            nc.sync.dma_start(out=w2t[:], in_=w_in[1][:, :])
            w3t = const.tile([DIMS[2], P], bf16, tag="w3")
            nc.sync.dma_start(out=w3t[:], in_=w_in[2][:, :])
            bt = []
            for i in range(3):
                t = const.tile([P, DIMS[i + 1]], f32, tag=f"b{i}", name=f"bt{i}")
                nc.sync.dma_start(out=t[:], in_=b_in[i][:, :])
                bt.append(t)
            selft = [selfp.tile([P, NBLK, DIMS[i + 1]], bf16, tag=f"self{i}",
                                name=f"self{i}") for i in range(3)]

            slabs = [dram.tile([SLABROWS, GD], bf16, tag=f"slab{i}", name=f"slab{i}") for i in range(3)]
            hbufs = [[dram.tile([CLSROWS, GD], bf16, tag=f"hbuf{i}_{q}",
                                name=f"hbuf{i}_{q}", addr_space="Shared")
                      for q in range(NQ)] for i in range(3)]
            for l in range(3):
                for q in range(NQ):
                    zr = q * CROWS + P * CLSBLK
                    nc.sync.dma_start(out=slabs[l][zr:zr + 2, :], in_=zt[0:2, :])

            def allgather(layer, q):
                nc.gpsimd.collective_compute(
                    "AllGather", AL.bypass,
                    replica_groups=[list(range(NCORES))],
                    ins=[slabs[layer][q * CROWS:(q + 1) * CROWS, :].opt()],
                    outs=[hbufs[layer][q][:, :].opt()])

            def slab_view(layer, q):
                a = q * CROWS
                v = slabs[layer][a:a + P * CLSBLK, :]
                return v.rearrange("(p t) d -> p t d", p=P)

            def issue_gathers(layer):
                tiles = {}
                for cid, (q, blks, cols) in enumerate(calls):
                    gt = gpool.tile([P, cols, GD], bf16, tag="gt")
                    nidx = cols * P
                    nc.gpsimd.dma_gather(
                        out_ap=gt[:, :, :],
                        in_ap=hbufs[layer][q][MID:2 * MID, :],
                        idxs_ap=idx_t[:, int(cbase[cid]) // 16:(int(cbase[cid]) + nidx) // 16],
                        num_idxs=nidx, num_idxs_reg=nidx, elem_size=GD,
                        single_packet=False, queue_num=cid % 4)
                    tiles[cid] = gt
                return tiles

            def reduce_block(tiles, j, dout, out_ap):
                # out_ap [P, dout] f32 = (sum of gathered) * dinv_j + self_j
                parts = []
                for q in range(NQ):
                    if blkq[j][q] is None:
                        continue
                    cid, off, kq = blkq[j][q]
                    view = tiles[cid][:, off:off + kq, :dout].rearrange("p k d -> p d k")
                    r = work.tile([P, dout], f32, tag=f"r{dout}")
                    nc.vector.tensor_reduce(out=r[:], in_=view,
                                            axis=mybir.AxisListType.X, op=AL.add)
                    parts.append(r)
                assert parts
                r = parts[0]
                if len(parts) == 2:
                    nc.vector.scalar_tensor_tensor(
                        out=r[:], in0=r[:], scalar=1.0, in1=parts[1][:],
                        op0=AL.mult, op1=AL.add)
                nc.vector.scalar_tensor_tensor(
                    out=out_ap, in0=r[:], scalar=dinv_t[:, j:j + 1],
                    in1=selft[0][:, j, :dout] if dout == DIMS[1]
                    else (selft[1][:, j, :dout] if dout == DIMS[2] else selft[2][:, j, :dout]),
                    op0=AL.mult, op1=AL.add)

            def transform_tail(i, j, b, src_ap, htb):
                # layer (i+2) transform of block j from pre-relu src [P, DIMS[i+1]]
                din = DIMS[i + 1]
                pt = ppt.tile([P, P], f32, space="PSUM", tag="trp")
                nc.tensor.transpose(out=pt[:din, :], in_=src_ap, identity=ident[:])
                sT = work.tile([din, P], bf16, tag=f"sT{din}")
                nc.scalar.activation(out=sT[:], in_=pt[:din, :],
                                     func=mybir.ActivationFunctionType.Relu)
                ps = pps.tile([P, P], f32, space="PSUM", tag="tps")
                wt = w2t if i == 0 else w3t
                nc.tensor.matmul(out=ps[:], lhsT=sT[:], rhs=wt[:], start=True, stop=True)
                nc.scalar.activation(out=htb[:, b, :], in_=ps[:],
                                     func=mybir.ActivationFunctionType.Copy,
                                     scale=dinv_t[:, j:j + 1])
                dnext = DIMS[i + 2]
                nc.vector.scalar_tensor_tensor(
                    out=selft[i + 1][:, j, :], in0=ps[:, :dnext],
                    scalar=dinv2_t[:, j:j + 1], in1=bt[i + 1][:],
                    op0=AL.mult, op1=AL.add)

            # ---- L1 transform: slab0 = dinv * (x @ W1); self0 = dinv^2*ps + b1
            xview = xT_in.rearrange("(k p) n -> p k n", p=P)
            for q in range(NQ):
                B0, B1 = q * CLSBLK, (q + 1) * CLSBLK
                for j0 in range(B0, B1, WB):
                    bsz = min(WB, B1 - j0)
                    lt = lhs.tile([P, 4, bsz * P], bf16, tag=f"xT{bsz}")
                    nc.sync.dma_start(out=lt[:], in_=xview[:, :, j0 * P:(j0 + bsz) * P])
                    htb = hbt.tile([P, bsz, GD], bf16, tag=f"htb{bsz}")
                    for b in range(bsz):
                        j = j0 + b
                        ps = pps.tile([P, P], f32, space="PSUM", tag="tps")
                        for k in range(4):
                            nc.tensor.matmul(out=ps[:], lhsT=lt[:, k, b * P:(b + 1) * P],
                                             rhs=w1t[k][:], start=(k == 0), stop=(k == 3))
                        nc.scalar.activation(out=htb[:, b, :], in_=ps[:],
                                             func=mybir.ActivationFunctionType.Copy,
                                             scale=dinv_t[:, j:j + 1])
                        nc.vector.scalar_tensor_tensor(
                            out=selft[0][:, j, :], in0=ps[:],
                            scalar=dinv2_t[:, j:j + 1], in1=bt[0][:],
                            op0=AL.mult, op1=AL.add)
                    nc.sync.dma_start(out=slab_view(0, q)[:, j0 - B0:j0 - B0 + bsz, :],
                                      in_=htb[:])
                allgather(0, q)

            # ---- agg layer i consumes hbufs[i], transforms into slabs[i+1]
            outv = out_t.rearrange("(p t) d -> p t d", p=P)
            for i in range(3):
                dout = DIMS[i + 1]
                tiles = issue_gathers(i)
                for q in range(NQ):
                    B0, B1 = q * CLSBLK, (q + 1) * CLSBLK
                    for j0 in range(B0, B1, WB):
                        bsz = min(WB, B1 - j0)
                        if i < 2:
                            htb = hbt.tile([P, bsz, GD], bf16, tag=f"htb{bsz}")
                        else:
                            htb = hbt.tile([P, bsz, DIMS[3]], f32, tag=f"ob{bsz}")
                        for b in range(bsz):
                            j = j0 + b
                            if i < 2:
                                s = work.tile([P, dout], f32, tag=f"s{dout}")
                                reduce_block(tiles, j, dout, s[:])
                                transform_tail(i, j, b, s[:], htb)
                            else:
                                reduce_block(tiles, j, dout, htb[:, b, :])
                        if i < 2:
                            nc.sync.dma_start(
                                out=slab_view(i + 1, q)[:, j0 - B0:j0 - B0 + bsz, :],
                                in_=htb[:])
                        else:
                            nc.sync.dma_start(out=outv[:, j0:j0 + bsz, :], in_=htb[:])
                    if i < 2:
                        allgather(i + 1, q)
    nc.compile()
    return nc


# --------------------------------------------------------------------------
# SPMD runner (shard_map over 8 axon cores, reusable jitted executable)
# --------------------------------------------------------------------------
class _Runner:
    def __init__(self, nc, n_cores=NCORES):
        import jax
        from jax.sharding import Mesh, PartitionSpec
        from jax.experimental.shard_map import shard_map
        from concourse import bass2jax, mybir
        bass2jax.install_neuronx_cc_hook()
        self.jax = jax
        self.n_cores = n_cores
        pname = nc.partition_id_tensor.name if nc.partition_id_tensor else None
        in_names, out_names, out_avals, zero_outs = [], [], [], []
        for alloc in nc.m.functions[0].allocations:
            if not isinstance(alloc, mybir.MemoryLocationSet):
                continue
            name = alloc.memorylocations[0].name
            if alloc.kind == "ExternalInput":
                if name != pname:
                    in_names.append(name)
            elif alloc.kind == "ExternalOutput":
                out_names.append(name)
                out_avals.append(jax.core.ShapedArray(tuple(alloc.tensor_shape), mybir.dt.np(alloc.dtype)))
                zero_outs.append(np.zeros(tuple(alloc.tensor_shape), mybir.dt.np(alloc.dtype)))
        self.in_names, self.out_names = in_names, out_names
        self.out_avals, self.zero_outs = out_avals, zero_outs
        n_params, n_outs = len(in_names), len(out_names)
        all_in = in_names + out_names + ([pname] if pname else [])

        def _body(*args):
            operands = list(args)
            if pname:
                operands.append(bass2jax.partition_id_tensor())
            outs = bass2jax._bass_exec_p.bind(
                *operands, out_avals=tuple(out_avals), in_names=tuple(all_in),
                out_names=tuple(out_names), lowering_input_output_aliases=(),
                sim_require_finite=True, sim_require_nnan=True, nc=nc)
            return tuple(outs)

        devices = jax.devices()[:n_cores]
        self.mesh = Mesh(np.asarray(devices), ("core",))
        self.pspec = PartitionSpec("core")
        self.fn = jax.jit(
            shard_map(_body, mesh=self.mesh,
                      in_specs=(self.pspec,) * (n_params + n_outs),
                      out_specs=(self.pspec,) * n_outs, check_rep=False),
            donate_argnums=tuple(range(n_params, n_params + n_outs)),
            keep_unused=True)

    def place(self, in_maps):
        sh = self.jax.sharding.NamedSharding(self.mesh, self.pspec)
        return [self.jax.device_put(
                    np.concatenate([np.asarray(in_maps[c][n]) for c in range(self.n_cores)], axis=0), sh)
                for n in self.in_names]

    def make_zeros(self):
        sh = self.jax.sharding.NamedSharding(self.mesh, self.pspec)
        zeros = [self.jax.device_put(
                    np.zeros((self.n_cores * z.shape[0], *z.shape[1:]), z.dtype), sh)
                 for z in self.zero_outs]
        self.jax.block_until_ready(zeros)
        return zeros

    def run(self, args, zeros=None):
        if zeros is None:
            zeros = self.make_zeros()
        outs = self.fn(*args, *zeros)
        self.jax.block_until_ready(outs)
        return outs

    def results(self, outs):
        return [{n: np.asarray(outs[i]).reshape(self.n_cores, *self.out_avals[i].shape)[c]
                 for i, n in enumerate(self.out_names)}
                for c in range(self.n_cores)]


# --------------------------------------------------------------------------
def _bf16(a):
    import ml_dtypes
    return np.asarray(a, dtype=ml_dtypes.bfloat16)


def build_in_maps(pre, x, W1, b1, W2, b2, W3, b3):
    ids = pre["ids"]                                  # [c, j, p] -> node
    x = np.asarray(x, np.float32)
    W2p = np.zeros((P, P), np.float32); W2p[:, :DIMS[2]] = np.asarray(W2, np.float32)
    W3p = np.zeros((DIMS[2], P), np.float32); W3p[:, :DIMS[3]] = np.asarray(W3, np.float32)
    in_maps = []
    for c in range(NCORES):
        v = ids[c].reshape(-1)                        # xT column j*128+p -> node
        xc = np.zeros((SLOTS, DIMS[0]), np.float32)
        m = v >= 0
        xc[m] = x[v[m]]
        in_maps.append({
            "xT": _bf16(np.ascontiguousarray(xc.T)),
            "gidx": pre["wrapped"][c],
            "dinv": pre["dinv_t"][c],
            "dinv2": pre["dinv2_t"][c],
            "W1": _bf16(np.asarray(W1, np.float32)),
            "W2": _bf16(W2p),
            "W3": _bf16(W3p),
            "b1": np.tile(np.asarray(b1, np.float32)[None, :], (P, 1)),
            "b2": np.tile(np.asarray(b2, np.float32)[None, :], (P, 1)),
            "b3": np.tile(np.asarray(b3, np.float32)[None, :], (P, 1)),
        })
    return in_maps


def _get(edge_index):
    key = hash(np.asarray(edge_index)[:, ::997].tobytes())
    if key not in _CACHE:
        pre = _preprocess(edge_index)
        nc = _build(pre)
        _CACHE[key] = (pre, _Runner(nc))
    return _CACHE[key]


def kernel(x, edge_index, W1, b1, W2, b2, W3, b3):
    pre, runner = _get(edge_index)
    in_maps = build_in_maps(pre, x, W1, b1, W2, b2, W3, b3)
    args = runner.place(in_maps)
    outs = runner.run(args)
    res = runner.results(outs)
    ids = pre["ids"]
    full = np.zeros((N, DIMS[3]), np.float32)
    for c in range(NCORES):
        v = ids[c]                                    # [j, p]
        # out row = p*NBLK + j (lane-major)
        o = res[c]["out"].reshape(P, NBLK, DIMS[3])
        m = v >= 0
        jj, pp = np.nonzero(m)
        full[v[jj, pp]] = o[pp, jj]
    return full


# revision 21
# speedup vs baseline: 1.3329x; 1.3329x over previous
"""3-layer GCN (PyG GCNConv x3, relu between) on 8 Trainium2 NeuronCores.

Math: out = A*(relu(A*(relu(A*(xW1)+b1)W2+b2))W3)+b3 with A = D^-1/2(A+I)D^-1/2.
Per layer: htilde = dinv * (input @ W) (dense, PE, bf16) is written to a
per-core slab and AllGathered per class (2 tables of 50,304 rows; class-0's
AG fires at the producer loop's midpoint, overlapping its second half);
aggregation of the REAL edges is a dma_gather + per-block strided
tensor_reduce; the self-loop term dinv^2*(input@W)[dst] + b is kept in SBUF
from the transform phase and added via one scalar_tensor_tensor, so
self-loops never transit the gather path.

Gather indexing: dma_gather takes int16 row indices. Each call's in_ap is
based at the middle of its class table, idx = row - MID in [-25152, 25151]
(HW sign-extends; only a negative index in the *final* list position is
dropped, so pads are positive and each call's last entry is forced
non-negative by an in-lane swap). Host preprocessing greedily 2-colors SRC
nodes (symmetric balance objective + 6 refinement sweeps) to split every
dst's in-edges evenly across the classes, then packs dsts with similar
(deg, class-0 count) into the same 128-lane block (snake order over (d, c0)
so block boundaries don't mix low-c0 and high-c0 extremes). Consecutive
blocks of one class share a single gather call (up to 4096 idxs) to
amortize the ~1us SWDGE fixed cost; class-1 calls are staggered after
class-0 ones so the in-order gpsimd queue isn't blocked by the class-1 AG.
Pad entries cycle over 8*ZPAD distinct zero rows -- descriptors that all
hit one row serialize at the HBM bank (~13.5 ns each, measured).

Layout: dst slot (core, block j, lane p). xT columns are block-major
(j*128+p); table/slab rows are lane-major within each class
(p*CLSBLK + j - q*CLSBLK) so 8-block batches of transform output form one
contiguous-per-partition DMA; out rows are lane-major global (p*98+j).
Each (core, class) appends ZPAD zero slots (gather pad targets).
"""
import sys
sys.path.insert(0, "/opt/trn_rl_repo")
import numpy as np

N = 100_000
DIMS = [512, 128, 64, 32]
NCORES = 8
P = 128
NBLK = 98
GD = 128
SLOTS = NBLK * P                 # 12544 xT columns / out rows per core
NQ = 2
CLSBLK = NBLK // NQ              # 49 blocks per class
ZPAD = 16                        # zero slots per (core, class): pad targets
                                 # spread over 8*ZPAD rows -- same-row gathers
                                 # serialize at the HBM bank (~13.5 ns/desc)
CROWS = CLSBLK * P + ZPAD        # per-core rows per class
SLABROWS = NQ * CROWS            # 12576
CLSROWS = NCORES * CROWS         # 50304 rows per class table
MID = CLSROWS // 2               # 25152
CALL_CAP_COLS = 32               # 4096 idxs per gather call
WB = 8                           # transform write batch (blocks)

_CACHE = {}


# --------------------------------------------------------------------------
# host-side graph preprocessing
# --------------------------------------------------------------------------
def _preprocess(edge_index):
    src = np.asarray(edge_index[0], np.int64)
    dst = np.asarray(edge_index[1], np.int64)
    indeg = np.bincount(dst, minlength=N).astype(np.int64)
    deg = indeg + 1                                   # + self loop (norm)
    dinv = (1.0 / np.sqrt(deg)).astype(np.float32)

    # ---- 2-coloring of SRC nodes: balance each dst's in-edges across halves
    order = np.argsort(src, kind="stable")
    S_srt, D_srt = src[order], dst[order]
    indptr = np.searchsorted(S_srt, np.arange(N + 1))
    M = np.zeros((N, NQ), np.int32)
    cls = np.full(N, -1, np.int8)
    capq = np.full(NQ, 49 * 1024, np.int64)
    proc = np.argsort(-(indptr[1:] - indptr[:-1]), kind="stable")
    for n in proc:
        nb = D_srt[indptr[n]:indptr[n + 1]]
        diff = int((M[nb, 0] - M[nb, 1]).sum())
        q = 0 if diff < 0 else 1
        if capq[q] <= 0:
            q = 1 - q
        cls[n] = q
        capq[q] -= 1
        M[nb, q] += 1
    for _ in range(6):
        moved = 0
        for n in proc:
            nb = D_srt[indptr[n]:indptr[n + 1]]
            q0 = cls[n]
            M[nb, q0] -= 1
            capq[q0] += 1
            diff = int((M[nb, 0] - M[nb, 1]).sum())
            q = 0 if diff < 0 else 1
            if capq[q] <= 0:
                q = 1 - q
            moved += q != q0
            cls[n] = q
            capq[q] -= 1
            M[nb, q] += 1
        if moved == 0:
            break

    # ---- pack dsts into blocks (snake order on (deg, c0)); class q -> its
    # 49 blocks; within a block, position t -> core t//128, lane t%128
    core_of = np.full(N, -1, np.int64)
    blk_of = np.full(N, -1, np.int64)
    lane_of = np.full(N, -1, np.int64)
    for q in range(NQ):
        nodes_q = np.flatnonzero(cls == q)
        c0 = M[nodes_q, 0].astype(np.int64)
        d = c0 + M[nodes_q, 1]
        key2 = np.where(d % 2 == 0, c0, 100000 - c0)
        stream = nodes_q[np.lexsort((key2, d))]
        jbase = q * CLSBLK
        for b in range(CLSBLK):
            seg = stream[b * 1024:(b + 1) * 1024]
            t = np.arange(len(seg))
            core_of[seg] = t // P
            blk_of[seg] = jbase + b
            lane_of[seg] = t % P

    # ---- table rows: class tensor row = c*CROWS + p*CLSBLK + (j - q*CLSBLK)
    ec, ej, el = core_of[dst], blk_of[dst], lane_of[dst]
    eq = cls[src].astype(np.int64)
    srow = (core_of[src] * CROWS + lane_of[src] * CLSBLK
            + (blk_of[src] - eq * CLSBLK))
    eidx = srow - MID
    assert eidx.min() >= -MID and eidx.max() < MID

    # per-(core, class, block, lane) counts -> K[q, j]
    key = ((ec * NQ + eq) * NBLK + ej) * P + el
    cnt = np.zeros(NCORES * NQ * NBLK * P, np.int64)
    np.add.at(cnt, key, 1)
    cnt4 = cnt.reshape(NCORES, NQ, NBLK, P)
    K = cnt4.max(axis=(0, 3))                         # [NQ, NBLK]

    order_e = np.argsort(key, kind="stable")
    ks = key[order_e]
    first = np.flatnonzero(np.r_[True, ks[1:] != ks[:-1]])
    within = np.arange(len(ks)) - first[np.searchsorted(ks[first], ks)]
    oc, oq, oj, ol = ec[order_e], eq[order_e], ej[order_e], el[order_e]
    oidx = eidx[order_e]

    # pad targets: cycle over the upper-half zero rows (cores 4..7, all
    # positive idx so any of them can legally terminate a call)
    pad_pool = np.array([c * CROWS + CLSBLK * P + t - MID
                         for c in range(4, 8) for t in range(ZPAD)], np.int64)
    assert (pad_pool >= 0).all() and pad_pool.max() < MID

    def build(K):
        # calls: per class, consecutive blocks, <= CALL_CAP_COLS columns
        calls = []                                    # (q, [(j, off, K)], cols)
        for q in range(NQ):
            cur, cols = [], 0
            for j in range(NBLK):
                kqj = int(K[q, j])
                if kqj == 0:
                    continue
                if cols + kqj > CALL_CAP_COLS and cur:
                    calls.append((q, cur, cols))
                    cur, cols = [], 0
                cur.append((j, cols, kqj))
                cols += kqj
            if cur:
                calls.append((q, cur, cols))
        # class-1 calls are staggered ~10 blocks later in issue order: the
        # gpsimd queue is in-order, and the first class-1 call blocks on the
        # class-1 AllGather mesh; meanwhile class-0 calls (whose AG completed
        # mid-producer-loop) keep the DMA engines fed.
        calls.sort(key=lambda cc: (cc[1][0][0] + (10 if cc[0] == 1 else 0), cc[0]))
        cbase = np.zeros(len(calls), np.int64)
        acc = 0
        cidm = np.full((NQ, NBLK), -1, np.int64)
        offm = np.zeros((NQ, NBLK), np.int64)
        blkq = [[None] * NQ for _ in range(NBLK)]
        for cid, (q, blks, cols) in enumerate(calls):
            cbase[cid] = acc
            acc += cols * P
            for (j, off, kqj) in blks:
                cidm[q, j], offm[q, j] = cid, off
                blkq[j][q] = (cid, off, kqj)
        total_idx = acc

        req = np.empty((NCORES, total_idx), np.int64)
        for c in range(NCORES):
            req[c] = pad_pool[(np.arange(total_idx) + 17 * c) % len(pad_pool)]
        ecall = cidm[oq, oj]
        ekoff = offm[oq, oj] + within
        req[oc, cbase[ecall] + ekoff * P + ol] = oidx
        return calls, cbase, blkq, total_idx, req

    for _ in range(10):
        calls, cbase, blkq, total_idx, req = build(K)
        bumped = False
        for cid, (q, blks, cols) in enumerate(calls):
            e = cbase[cid] + cols * P - 1
            jl, offl, kl = blks[-1]
            # final entry = lane 127, top-k of the call's last block; if
            # negative, swap with a non-negative entry in the same lane/block
            # k-range (order within a k-range is irrelevant to the sum)
            cols_l = cbase[cid] + (offl + np.arange(kl)) * P + (P - 1)
            for c in range(NCORES):
                if req[c, e] < 0:
                    cand = np.flatnonzero(req[c, cols_l] >= 0)
                    if len(cand) == 0:
                        K[q, jl] += 1
                        bumped = True
                        break
                    s_ = cols_l[cand[0]]
                    req[c, e], req[c, s_] = req[c, s_], req[c, e]
            if bumped:
                break
        if not bumped:
            break
    assert req.max() < 32768 and req.min() >= -32768

    wrapped = np.empty((NCORES, P, total_idx // 16), np.int16)
    for c in range(NCORES):
        col = 0
        for cid, (q, blks, cols) in enumerate(calls):
            L = req[c, cbase[cid]:cbase[cid] + cols * P]
            w = L.reshape(-1, 16).T.astype(np.int16)
            wrapped[c, :, col:col + cols * P // 16] = np.tile(w, (8, 1))
            col += cols * P // 16

    # ---- per-core node placement tables
    ids = np.full((NCORES, NBLK, P), -1, np.int64)    # [c, j, p] -> node
    m = core_of >= 0
    ids[core_of[m], blk_of[m], lane_of[m]] = np.flatnonzero(m)
    dinv_t = np.zeros((NCORES, P, NBLK), np.float32)
    dinv2_t = np.zeros((NCORES, P, NBLK), np.float32)
    for c in range(NCORES):
        v = ids[c]                                    # [j, p]
        dv = np.where(v >= 0, dinv[np.maximum(v, 0)], 0.0).astype(np.float32)
        dinv_t[c] = dv.T
        dinv2_t[c] = (dv * dv).T
    return dict(ids=ids, calls=calls, cbase=cbase, blkq=blkq,
                wrapped=wrapped, dinv_t=dinv_t, dinv2_t=dinv2_t,
                total_idx=total_idx)


# --------------------------------------------------------------------------
# bass program
# --------------------------------------------------------------------------
def _build(pre, reps=1, nq_dma=4):
    from concourse import bass, bacc, mybir, tile
    from concourse.library_config import mlp
    from concourse.masks import make_identity
    AL = mybir.AluOpType
    f32, bf16, i16 = mybir.dt.float32, mybir.dt.bfloat16, mybir.dt.int16
    calls, blkq, cbase = pre["calls"], pre["blkq"], pre["cbase"]
    total_idx = pre["total_idx"]

    nc = bacc.Bacc("TRN2", target_bir_lowering=False, debug=False,
                   num_devices=NCORES, num_swdge_queues=nq_dma)
    xT_in = nc.dram_tensor("xT", (DIMS[0], SLOTS), bf16, kind="ExternalInput")
    idx_in = nc.dram_tensor("gidx", (P, total_idx // 16), i16, kind="ExternalInput")
    dinv_in = nc.dram_tensor("dinv", (P, NBLK), f32, kind="ExternalInput")
    dinv2_in = nc.dram_tensor("dinv2", (P, NBLK), f32, kind="ExternalInput")
    w_in = [nc.dram_tensor("W1", (DIMS[0], P), bf16, kind="ExternalInput"),
            nc.dram_tensor("W2", (P, P), bf16, kind="ExternalInput"),
            nc.dram_tensor("W3", (DIMS[2], P), bf16, kind="ExternalInput")]
    b_in = [nc.dram_tensor(f"b{i+1}", (P, DIMS[i + 1]), f32, kind="ExternalInput")
            for i in range(3)]
    out_t = nc.dram_tensor("out", (SLOTS, DIMS[3]), f32, kind="ExternalOutput")

    with tile.TileContext(nc) as tc:
        with tc.tile_pool(name="const", bufs=1) as const, \
             tc.tile_pool(name="selfp", bufs=1) as selfp, \
             tc.tile_pool(name="gath", bufs=10) as gpool, \
             tc.tile_pool(name="work", bufs=4) as work, \
             tc.tile_pool(name="hbt", bufs=3) as hbt, \
             tc.tile_pool(name="lhs", bufs=2) as lhs, \
             tc.tile_pool(name="pps", bufs=2, space="PSUM") as pps, \
             tc.tile_pool(name="ppt", bufs=2, space="PSUM") as ppt, \
             tc.tile_pool(name="dram", bufs=1, space="DRAM") as dram:

            nc.gpsimd.load_library(mlp)
            idx_t = const.tile([P, total_idx // 16], i16, tag="idx")
            nc.sync.dma_start(out=idx_t[:], in_=idx_in[:, :])
            dinv_t = const.tile([P, NBLK], f32, tag="dinv")
            nc.sync.dma_start(out=dinv_t[:], in_=dinv_in[:, :])
            dinv2_t = const.tile([P, NBLK], f32, tag="dinv2")
            nc.sync.dma_start(out=dinv2_t[:], in_=dinv2_in[:, :])
            ident = const.tile([P, P], f32, tag="ident")
            make_identity(nc, ident[:])
            zt = const.tile([P, GD], bf16, tag="zero")
            nc.vector.memset(zt[:], 0.0)
            w1t = [const.tile([P, P], bf16, tag=f"w1_{k}", name=f"w1_{k}") for k in range(4)]
            for k in range(4):
                nc.sync.dma_start(out=w1t[k][:], in_=w_in[0][k * P:(k + 1) * P, :])
            w2t = const.tile([P, P], bf16, tag="w2")
            nc.sync.dma_start(out=w2t[:], in_=w_in[1][:, :])
            w3t = const.tile([DIMS[2], P], bf16, tag="w3")
            nc.sync.dma_start(out=w3t[:], in_=w_in[2][:, :])
            bt = []
            for i in range(3):
                t = const.tile([P, DIMS[i + 1]], f32, tag=f"b{i}", name=f"bt{i}")
                nc.sync.dma_start(out=t[:], in_=b_in[i][:, :])
                bt.append(t)
            def one_rep(rep):
                selft = [selfp.tile([P, NBLK, DIMS[i + 1]], bf16, tag=f"self{i}",
                                    name=f"self{i}_r{rep}") for i in range(3)]
                slabs = [dram.tile([SLABROWS, GD], bf16, tag=f"slab{i}",
                                   name=f"slab{i}_r{rep}") for i in range(3)]
                hbufs = [[dram.tile([CLSROWS, GD], bf16, tag=f"hbuf{i}_{q}",
                                    name=f"hbuf{i}_{q}_r{rep}", addr_space="Shared")
                          for q in range(NQ)] for i in range(3)]
                for l in range(3):
                    for q in range(NQ):
                        zr = q * CROWS + P * CLSBLK
                        nc.sync.dma_start(out=slabs[l][zr:zr + ZPAD, :], in_=zt[0:ZPAD, :])

            def allgather(layer, q):
                nc.gpsimd.collective_compute(
                    "AllGather", AL.bypass,
                    replica_groups=[list(range(NCORES))],
                    ins=[slabs[layer][q * CROWS:(q + 1) * CROWS, :].opt()],
                    outs=[hbufs[layer][q][:, :].opt()])

            def slab_view(layer, q):
                a = q * CROWS
                v = slabs[layer][a:a + P * CLSBLK, :]
                return v.rearrange("(p t) d -> p t d", p=P)

            def issue_gathers(layer):
                tiles = {}
                for cid, (q, blks, cols) in enumerate(calls):
                    gt = gpool.tile([P, cols, GD], bf16, tag="gt")
                    nidx = cols * P
                    nc.gpsimd.dma_gather(
                        out_ap=gt[:, :, :],
                        in_ap=hbufs[layer][q][MID:2 * MID, :],
                        idxs_ap=idx_t[:, int(cbase[cid]) // 16:(int(cbase[cid]) + nidx) // 16],
                        num_idxs=nidx, num_idxs_reg=nidx, elem_size=GD,
                        single_packet=False, queue_num=cid % nq_dma)
                    tiles[cid] = gt
                return tiles

            def reduce_block(tiles, j, dout, out_ap):
                # out_ap [P, dout] f32 = (sum of gathered) * dinv_j + self_j
                parts = []
                for q in range(NQ):
                    if blkq[j][q] is None:
                        continue
                    cid, off, kq = blkq[j][q]
                    view = tiles[cid][:, off:off + kq, :dout].rearrange("p k d -> p d k")
                    r = work.tile([P, dout], f32, tag=f"r{dout}")
                    nc.vector.tensor_reduce(out=r[:], in_=view,
                                            axis=mybir.AxisListType.X, op=AL.add)
                    parts.append(r)
                assert parts
                r = parts[0]
                if len(parts) == 2:
                    nc.vector.scalar_tensor_tensor(
                        out=r[:], in0=r[:], scalar=1.0, in1=parts[1][:],
                        op0=AL.mult, op1=AL.add)
                nc.vector.scalar_tensor_tensor(
                    out=out_ap, in0=r[:], scalar=dinv_t[:, j:j + 1],
                    in1=selft[0][:, j, :dout] if dout == DIMS[1]
                    else (selft[1][:, j, :dout] if dout == DIMS[2] else selft[2][:, j, :dout]),
                    op0=AL.mult, op1=AL.add)

            def transform_tail(i, j, b, src_ap, htb):
                # layer (i+2) transform of block j from pre-relu src [P, DIMS[i+1]]
                din = DIMS[i + 1]
                pt = ppt.tile([P, P], f32, space="PSUM", tag="trp")
                nc.tensor.transpose(out=pt[:din, :], in_=src_ap, identity=ident[:])
                sT = work.tile([din, P], bf16, tag=f"sT{din}")
                nc.scalar.activation(out=sT[:], in_=pt[:din, :],
                                     func=mybir.ActivationFunctionType.Relu)
                ps = pps.tile([P, P], f32, space="PSUM", tag="tps")
                wt = w2t if i == 0 else w3t
                nc.tensor.matmul(out=ps[:], lhsT=sT[:], rhs=wt[:], start=True, stop=True)
                nc.scalar.activation(out=htb[:, b, :], in_=ps[:],
                                     func=mybir.ActivationFunctionType.Copy,
                                     scale=dinv_t[:, j:j + 1])
                dnext = DIMS[i + 2]
                nc.vector.scalar_tensor_tensor(
                    out=selft[i + 1][:, j, :], in0=ps[:, :dnext],
                    scalar=dinv2_t[:, j:j + 1], in1=bt[i + 1][:],
                    op0=AL.mult, op1=AL.add)

            # ---- L1 transform: slab0 = dinv * (x @ W1); self0 = dinv^2*ps + b1
            xview = xT_in.rearrange("(k p) n -> p k n", p=P)
            for q in range(NQ):
                B0, B1 = q * CLSBLK, (q + 1) * CLSBLK
                for j0 in range(B0, B1, WB):
                    bsz = min(WB, B1 - j0)
                    lt = lhs.tile([P, 4, bsz * P], bf16, tag=f"xT{bsz}")
                    nc.sync.dma_start(out=lt[:], in_=xview[:, :, j0 * P:(j0 + bsz) * P])
                    htb = hbt.tile([P, bsz, GD], bf16, tag=f"htb{bsz}")
                    for b in range(bsz):
                        j = j0 + b
                        ps = pps.tile([P, P], f32, space="PSUM", tag="tps")
                        for k in range(4):
                            nc.tensor.matmul(out=ps[:], lhsT=lt[:, k, b * P:(b + 1) * P],
                                             rhs=w1t[k][:], start=(k == 0), stop=(k == 3))
                        nc.scalar.activation(out=htb[:, b, :], in_=ps[:],
                                             func=mybir.ActivationFunctionType.Copy,
                                             scale=dinv_t[:, j:j + 1])
                        nc.vector.scalar_tensor_tensor(
                            out=selft[0][:, j, :], in0=ps[:],
                            scalar=dinv2_t[:, j:j + 1], in1=bt[0][:],
                            op0=AL.mult, op1=AL.add)
                    nc.sync.dma_start(out=slab_view(0, q)[:, j0 - B0:j0 - B0 + bsz, :],
                                      in_=htb[:])
                allgather(0, q)

            # ---- agg layer i consumes hbufs[i], transforms into slabs[i+1]
            outv = out_t.rearrange("(p t) d -> p t d", p=P)
            for i in range(3):
                dout = DIMS[i + 1]
                tiles = issue_gathers(i)
                for q in range(NQ):
                    B0, B1 = q * CLSBLK, (q + 1) * CLSBLK
                    for j0 in range(B0, B1, WB):
                        bsz = min(WB, B1 - j0)
                        if i < 2:
                            htb = hbt.tile([P, bsz, GD], bf16, tag=f"htb{bsz}")
                        else:
                            htb = hbt.tile([P, bsz, DIMS[3]], f32, tag=f"ob{bsz}")
                        for b in range(bsz):
                            j = j0 + b
                            if i < 2:
                                s = work.tile([P, dout], f32, tag=f"s{dout}")
                                reduce_block(tiles, j, dout, s[:])
                                transform_tail(i, j, b, s[:], htb)
                            else:
                                reduce_block(tiles, j, dout, htb[:, b, :])
                        if i < 2:
                            nc.sync.dma_start(
                                out=slab_view(i + 1, q)[:, j0 - B0:j0 - B0 + bsz, :],
                                in_=htb[:])
                        else:
                            nc.sync.dma_start(out=outv[:, j0:j0 + bsz, :], in_=htb[:])
                    if i < 2:
                        allgather(i + 1, q)
    nc.compile()
    return nc


# --------------------------------------------------------------------------
# SPMD runner (shard_map over 8 axon cores, reusable jitted executable)
# --------------------------------------------------------------------------
class _Runner:
    def __init__(self, nc, n_cores=NCORES):
        import jax
        from jax.sharding import Mesh, PartitionSpec
        from jax.experimental.shard_map import shard_map
        from concourse import bass2jax, mybir
        bass2jax.install_neuronx_cc_hook()
        self.jax = jax
        self.n_cores = n_cores
        pname = nc.partition_id_tensor.name if nc.partition_id_tensor else None
        in_names, out_names, out_avals, zero_outs = [], [], [], []
        for alloc in nc.m.functions[0].allocations:
            if not isinstance(alloc, mybir.MemoryLocationSet):
                continue
            name = alloc.memorylocations[0].name
            if alloc.kind == "ExternalInput":
                if name != pname:
                    in_names.append(name)
            elif alloc.kind == "ExternalOutput":
                out_names.append(name)
                out_avals.append(jax.core.ShapedArray(tuple(alloc.tensor_shape), mybir.dt.np(alloc.dtype)))
                zero_outs.append(np.zeros(tuple(alloc.tensor_shape), mybir.dt.np(alloc.dtype)))
        self.in_names, self.out_names = in_names, out_names
        self.out_avals, self.zero_outs = out_avals, zero_outs
        n_params, n_outs = len(in_names), len(out_names)
        all_in = in_names + out_names + ([pname] if pname else [])

        def _body(*args):
            operands = list(args)
            if pname:
                operands.append(bass2jax.partition_id_tensor())
            outs = bass2jax._bass_exec_p.bind(
                *operands, out_avals=tuple(out_avals), in_names=tuple(all_in),
                out_names=tuple(out_names), lowering_input_output_aliases=(),
                sim_require_finite=True, sim_require_nnan=True, nc=nc)
            return tuple(outs)

        devices = jax.devices()[:n_cores]
        self.mesh = Mesh(np.asarray(devices), ("core",))
        self.pspec = PartitionSpec("core")
        self.fn = jax.jit(
            shard_map(_body, mesh=self.mesh,
                      in_specs=(self.pspec,) * (n_params + n_outs),
                      out_specs=(self.pspec,) * n_outs, check_rep=False),
            donate_argnums=tuple(range(n_params, n_params + n_outs)),
            keep_unused=True)

    def place(self, in_maps):
        sh = self.jax.sharding.NamedSharding(self.mesh, self.pspec)
        return [self.jax.device_put(
                    np.concatenate([np.asarray(in_maps[c][n]) for c in range(self.n_cores)], axis=0), sh)
                for n in self.in_names]

    def make_zeros(self):
        sh = self.jax.sharding.NamedSharding(self.mesh, self.pspec)
        zeros = [self.jax.device_put(
                    np.zeros((self.n_cores * z.shape[0], *z.shape[1:]), z.dtype), sh)
                 for z in self.zero_outs]
        self.jax.block_until_ready(zeros)
        return zeros

    def run(self, args, zeros=None):
        if zeros is None:
            zeros = self.make_zeros()
        outs = self.fn(*args, *zeros)
        self.jax.block_until_ready(outs)
        return outs

    def results(self, outs):
        return [{n: np.asarray(outs[i]).reshape(self.n_cores, *self.out_avals[i].shape)[c]
                 for i, n in enumerate(self.out_names)}
                for c in range(self.n_cores)]


# --------------------------------------------------------------------------
def _bf16(a):
    import ml_dtypes
    return np.asarray(a, dtype=ml_dtypes.bfloat16)


def build_in_maps(pre, x, W1, b1, W2, b2, W3, b3):
    ids = pre["ids"]                                  # [c, j, p] -> node
    x = np.asarray(x, np.float32)
    W2p = np.zeros((P, P), np.float32); W2p[:, :DIMS[2]] = np.asarray(W2, np.float32)
    W3p = np.zeros((DIMS[2], P), np.float32); W3p[:, :DIMS[3]] = np.asarray(W3, np.float32)
    in_maps = []
    for c in range(NCORES):
        v = ids[c].reshape(-1)                        # xT column j*128+p -> node
        xc = np.zeros((SLOTS, DIMS[0]), np.float32)
        m = v >= 0
        xc[m] = x[v[m]]
        in_maps.append({
            "xT": _bf16(np.ascontiguousarray(xc.T)),
            "gidx": pre["wrapped"][c],
            "dinv": pre["dinv_t"][c],
            "dinv2": pre["dinv2_t"][c],
            "W1": _bf16(np.asarray(W1, np.float32)),
            "W2": _bf16(W2p),
            "W3": _bf16(W3p),
            "b1": np.tile(np.asarray(b1, np.float32)[None, :], (P, 1)),
            "b2": np.tile(np.asarray(b2, np.float32)[None, :], (P, 1)),
            "b3": np.tile(np.asarray(b3, np.float32)[None, :], (P, 1)),
        })
    return in_maps


def _get(edge_index):
    key = hash(np.asarray(edge_index)[:, ::997].tobytes())
    if key not in _CACHE:
        pre = _preprocess(edge_index)
        nc = _build(pre)
        _CACHE[key] = (pre, _Runner(nc))
    return _CACHE[key]


def kernel(x, edge_index, W1, b1, W2, b2, W3, b3):
    pre, runner = _get(edge_index)
    in_maps = build_in_maps(pre, x, W1, b1, W2, b2, W3, b3)
    args = runner.place(in_maps)
    outs = runner.run(args)
    res = runner.results(outs)
    ids = pre["ids"]
    full = np.zeros((N, DIMS[3]), np.float32)
    for c in range(NCORES):
        v = ids[c]                                    # [j, p]
        # out row = p*NBLK + j (lane-major)
        o = res[c]["out"].reshape(P, NBLK, DIMS[3])
        m = v >= 0
        jj, pp = np.nonzero(m)
        full[v[jj, pp]] = o[pp, jj]
    return full


# revision 23
# speedup vs baseline: 1.5222x; 1.1420x over previous
"""3-layer GCN (PyG GCNConv x3, relu between) on 8 Trainium2 NeuronCores.

Math: out = A*(relu(A*(relu(A*(xW1)+b1)W2+b2))W3)+b3 with A = D^-1/2(A+I)D^-1/2.
Per layer: htilde = dinv * (input @ W) (dense, PE, bf16) is written to a
per-core slab and AllGathered per class (2 tables of 50,304 rows; class-0's
AG fires at the producer loop's midpoint, overlapping its second half);
aggregation of the REAL edges is a dma_gather + per-block strided
tensor_reduce; the self-loop term dinv^2*(input@W)[dst] + b is kept in SBUF
from the transform phase and added via one scalar_tensor_tensor, so
self-loops never transit the gather path.

Gather indexing: dma_gather takes int16 row indices. Each call's in_ap is
based at the middle of its class table, idx = row - MID in [-25152, 25151]
(HW sign-extends; only a negative index in the *final* list position is
dropped, so pads are positive and each call's last entry is forced
non-negative by an in-lane swap). Host preprocessing greedily 2-colors SRC
nodes (symmetric balance objective + 6 refinement sweeps) to split every
dst's in-edges evenly across the classes, then packs dsts with similar
(deg, class-0 count) into the same 128-lane block (snake order over (d, c0)
so block boundaries don't mix low-c0 and high-c0 extremes). Consecutive
blocks of one class share a single gather call (up to 4096 idxs) to
amortize the ~1us SWDGE fixed cost; class-1 calls are staggered after
class-0 ones so the in-order gpsimd queue isn't blocked by the class-1 AG.
Pad entries cycle over 8*ZPAD distinct zero rows -- descriptors that all
hit one row serialize at the HBM bank (~13.5 ns each, measured).

Layout: dst slot (core, block j, lane p). xT columns are block-major
(j*128+p); table/slab rows are lane-major within each class
(p*CLSBLK + j - q*CLSBLK) so 8-block batches of transform output form one
contiguous-per-partition DMA; out rows are lane-major global (p*98+j).
Each (core, class) appends ZPAD zero slots (gather pad targets).
"""
import sys
sys.path.insert(0, "/opt/trn_rl_repo")
import numpy as np

N = 100_000
DIMS = [512, 128, 64, 32]
NCORES = 8
P = 128
NBLK = 98
GD = 128
SLOTS = NBLK * P                 # 12544 xT columns / out rows per core
NQ = 2
CLSBLK = NBLK // NQ              # 49 blocks per class
ZPAD = 16                        # zero slots per (core, class): pad targets
                                 # spread over 8*ZPAD rows -- same-row gathers
                                 # serialize at the HBM bank (~13.5 ns/desc)
CROWS = CLSBLK * P + ZPAD        # per-core rows per class
SLABROWS = NQ * CROWS            # 12576
CLSROWS = NCORES * CROWS         # 50304 rows per class table
MID = CLSROWS // 2               # 25152
CALL_CAP_COLS = 32               # 4096 idxs per gather call
WB = 8                           # transform write batch (blocks)

_CACHE = {}


# --------------------------------------------------------------------------
# host-side graph preprocessing
# --------------------------------------------------------------------------
def _preprocess(edge_index):
    src = np.asarray(edge_index[0], np.int64)
    dst = np.asarray(edge_index[1], np.int64)
    indeg = np.bincount(dst, minlength=N).astype(np.int64)
    deg = indeg + 1                                   # + self loop (norm)
    dinv = (1.0 / np.sqrt(deg)).astype(np.float32)

    # ---- 2-coloring of SRC nodes: balance each dst's in-edges across halves
    order = np.argsort(src, kind="stable")
    S_srt, D_srt = src[order], dst[order]
    indptr = np.searchsorted(S_srt, np.arange(N + 1))
    M = np.zeros((N, NQ), np.int32)
    cls = np.full(N, -1, np.int8)
    capq = np.full(NQ, 49 * 1024, np.int64)
    proc = np.argsort(-(indptr[1:] - indptr[:-1]), kind="stable")
    for n in proc:
        nb = D_srt[indptr[n]:indptr[n + 1]]
        diff = int((M[nb, 0] - M[nb, 1]).sum())
        q = 0 if diff < 0 else 1
        if capq[q] <= 0:
            q = 1 - q
        cls[n] = q
        capq[q] -= 1
        M[nb, q] += 1
    for _ in range(6):
        moved = 0
        for n in proc:
            nb = D_srt[indptr[n]:indptr[n + 1]]
            q0 = cls[n]
            M[nb, q0] -= 1
            capq[q0] += 1
            diff = int((M[nb, 0] - M[nb, 1]).sum())
            q = 0 if diff < 0 else 1
            if capq[q] <= 0:
                q = 1 - q
            moved += q != q0
            cls[n] = q
            capq[q] -= 1
            M[nb, q] += 1
        if moved == 0:
            break

    # ---- pack dsts into blocks (snake order on (deg, c0)); class q -> its
    # 49 blocks; within a block, position t -> core t//128, lane t%128
    core_of = np.full(N, -1, np.int64)
    blk_of = np.full(N, -1, np.int64)
    lane_of = np.full(N, -1, np.int64)
    for q in range(NQ):
        nodes_q = np.flatnonzero(cls == q)
        c0 = M[nodes_q, 0].astype(np.int64)
        d = c0 + M[nodes_q, 1]
        key2 = np.where(d % 2 == 0, c0, 100000 - c0)
        stream = nodes_q[np.lexsort((key2, d))]
        jbase = q * CLSBLK
        for b in range(CLSBLK):
            seg = stream[b * 1024:(b + 1) * 1024]
            t = np.arange(len(seg))
            core_of[seg] = t // P
            blk_of[seg] = jbase + b
            lane_of[seg] = t % P

    # ---- table rows: class tensor row = c*CROWS + p*CLSBLK + (j - q*CLSBLK)
    ec, ej, el = core_of[dst], blk_of[dst], lane_of[dst]
    eq = cls[src].astype(np.int64)
    srow = (core_of[src] * CROWS + lane_of[src] * CLSBLK
            + (blk_of[src] - eq * CLSBLK))
    eidx = srow - MID
    assert eidx.min() >= -MID and eidx.max() < MID

    # per-(core, class, block, lane) counts -> K[q, j]
    key = ((ec * NQ + eq) * NBLK + ej) * P + el
    cnt = np.zeros(NCORES * NQ * NBLK * P, np.int64)
    np.add.at(cnt, key, 1)
    cnt4 = cnt.reshape(NCORES, NQ, NBLK, P)
    K = cnt4.max(axis=(0, 3))                         # [NQ, NBLK]

    order_e = np.argsort(key, kind="stable")
    ks = key[order_e]
    first = np.flatnonzero(np.r_[True, ks[1:] != ks[:-1]])
    within = np.arange(len(ks)) - first[np.searchsorted(ks[first], ks)]
    oc, oq, oj, ol = ec[order_e], eq[order_e], ej[order_e], el[order_e]
    oidx = eidx[order_e]

    # pad targets: cycle over the upper-half zero rows (cores 4..7, all
    # positive idx so any of them can legally terminate a call)
    pad_pool = np.array([c * CROWS + CLSBLK * P + t - MID
                         for c in range(4, 8) for t in range(ZPAD)], np.int64)
    assert (pad_pool >= 0).all() and pad_pool.max() < MID

    def build(K):
        # calls: per class, consecutive blocks, <= CALL_CAP_COLS columns
        calls = []                                    # (q, [(j, off, K)], cols)
        for q in range(NQ):
            cur, cols = [], 0
            for j in range(NBLK):
                kqj = int(K[q, j])
                if kqj == 0:
                    continue
                if cols + kqj > CALL_CAP_COLS and cur:
                    calls.append((q, cur, cols))
                    cur, cols = [], 0
                cur.append((j, cols, kqj))
                cols += kqj
            if cur:
                calls.append((q, cur, cols))
        # class-1 calls are staggered ~10 blocks later in issue order: the
        # gpsimd queue is in-order, and the first class-1 call blocks on the
        # class-1 AllGather mesh; meanwhile class-0 calls (whose AG completed
        # mid-producer-loop) keep the DMA engines fed.
        calls.sort(key=lambda cc: (cc[1][0][0] + (10 if cc[0] == 1 else 0), cc[0]))
        cbase = np.zeros(len(calls), np.int64)
        acc = 0
        cidm = np.full((NQ, NBLK), -1, np.int64)
        offm = np.zeros((NQ, NBLK), np.int64)
        blkq = [[None] * NQ for _ in range(NBLK)]
        for cid, (q, blks, cols) in enumerate(calls):
            cbase[cid] = acc
            acc += cols * P
            for (j, off, kqj) in blks:
                cidm[q, j], offm[q, j] = cid, off
                blkq[j][q] = (cid, off, kqj)
        total_idx = acc

        req = np.empty((NCORES, total_idx), np.int64)
        for c in range(NCORES):
            req[c] = pad_pool[(np.arange(total_idx) + 17 * c) % len(pad_pool)]
        ecall = cidm[oq, oj]
        ekoff = offm[oq, oj] + within
        req[oc, cbase[ecall] + ekoff * P + ol] = oidx
        return calls, cbase, blkq, total_idx, req

    for _ in range(10):
        calls, cbase, blkq, total_idx, req = build(K)
        bumped = False
        for cid, (q, blks, cols) in enumerate(calls):
            e = cbase[cid] + cols * P - 1
            jl, offl, kl = blks[-1]
            # final entry = lane 127, top-k of the call's last block; if
            # negative, swap with a non-negative entry in the same lane/block
            # k-range (order within a k-range is irrelevant to the sum)
            cols_l = cbase[cid] + (offl + np.arange(kl)) * P + (P - 1)
            for c in range(NCORES):
                if req[c, e] < 0:
                    cand = np.flatnonzero(req[c, cols_l] >= 0)
                    if len(cand) == 0:
                        K[q, jl] += 1
                        bumped = True
                        break
                    s_ = cols_l[cand[0]]
                    req[c, e], req[c, s_] = req[c, s_], req[c, e]
            if bumped:
                break
        if not bumped:
            break
    assert req.max() < 32768 and req.min() >= -32768

    wrapped = np.empty((NCORES, P, total_idx // 16), np.int16)
    for c in range(NCORES):
        col = 0
        for cid, (q, blks, cols) in enumerate(calls):
            L = req[c, cbase[cid]:cbase[cid] + cols * P]
            w = L.reshape(-1, 16).T.astype(np.int16)
            wrapped[c, :, col:col + cols * P // 16] = np.tile(w, (8, 1))
            col += cols * P // 16

    # ---- per-core node placement tables
    ids = np.full((NCORES, NBLK, P), -1, np.int64)    # [c, j, p] -> node
    m = core_of >= 0
    ids[core_of[m], blk_of[m], lane_of[m]] = np.flatnonzero(m)
    dinv_t = np.zeros((NCORES, P, NBLK), np.float32)
    dinv2_t = np.zeros((NCORES, P, NBLK), np.float32)
    for c in range(NCORES):
        v = ids[c]                                    # [j, p]
        dv = np.where(v >= 0, dinv[np.maximum(v, 0)], 0.0).astype(np.float32)
        dinv_t[c] = dv.T
        dinv2_t[c] = (dv * dv).T
    return dict(ids=ids, calls=calls, cbase=cbase, blkq=blkq,
                wrapped=wrapped, dinv_t=dinv_t, dinv2_t=dinv2_t,
                total_idx=total_idx)


# --------------------------------------------------------------------------
# bass program
# --------------------------------------------------------------------------
def _build(pre, reps=1, nq_dma=4):
    from concourse import bass, bacc, mybir, tile
    from concourse.library_config import mlp
    from concourse.masks import make_identity
    AL = mybir.AluOpType
    f32, bf16, i16 = mybir.dt.float32, mybir.dt.bfloat16, mybir.dt.int16
    calls, blkq, cbase = pre["calls"], pre["blkq"], pre["cbase"]
    total_idx = pre["total_idx"]

    nc = bacc.Bacc("TRN2", target_bir_lowering=False, debug=False,
                   num_devices=NCORES, num_swdge_queues=nq_dma)
    xT_in = nc.dram_tensor("xT", (DIMS[0], SLOTS), bf16, kind="ExternalInput")
    idx_in = nc.dram_tensor("gidx", (P, total_idx // 16), i16, kind="ExternalInput")
    dinv_in = nc.dram_tensor("dinv", (P, NBLK), f32, kind="ExternalInput")
    dinv2_in = nc.dram_tensor("dinv2", (P, NBLK), f32, kind="ExternalInput")
    w_in = [nc.dram_tensor("W1", (DIMS[0], P), bf16, kind="ExternalInput"),
            nc.dram_tensor("W2", (P, P), bf16, kind="ExternalInput"),
            nc.dram_tensor("W3", (DIMS[2], P), bf16, kind="ExternalInput")]
    b_in = [nc.dram_tensor(f"b{i+1}", (P, DIMS[i + 1]), f32, kind="ExternalInput")
            for i in range(3)]
    out_t = nc.dram_tensor("out", (SLOTS, DIMS[3]), f32, kind="ExternalOutput")

    with tile.TileContext(nc) as tc:
        with tc.tile_pool(name="const", bufs=1) as const, \
             tc.tile_pool(name="selfp", bufs=1) as selfp, \
             tc.tile_pool(name="gath", bufs=10) as gpool, \
             tc.tile_pool(name="work", bufs=4) as work, \
             tc.tile_pool(name="hbt", bufs=3) as hbt, \
             tc.tile_pool(name="lhs", bufs=2) as lhs, \
             tc.tile_pool(name="pps", bufs=2, space="PSUM") as pps, \
             tc.tile_pool(name="ppt", bufs=2, space="PSUM") as ppt, \
             tc.tile_pool(name="dram", bufs=1, space="DRAM") as dram:

            nc.gpsimd.load_library(mlp)
            idx_t = const.tile([P, total_idx // 16], i16, tag="idx")
            nc.sync.dma_start(out=idx_t[:], in_=idx_in[:, :])
            dinv_t = const.tile([P, NBLK], f32, tag="dinv")
            nc.sync.dma_start(out=dinv_t[:], in_=dinv_in[:, :])
            dinv2_t = const.tile([P, NBLK], f32, tag="dinv2")
            nc.sync.dma_start(out=dinv2_t[:], in_=dinv2_in[:, :])
            ident = const.tile([P, P], f32, tag="ident")
            make_identity(nc, ident[:])
            zt = const.tile([P, GD], bf16, tag="zero")
            nc.vector.memset(zt[:], 0.0)
            w1t = [const.tile([P, P], bf16, tag=f"w1_{k}", name=f"w1_{k}") for k in range(4)]
            for k in range(4):
                nc.sync.dma_start(out=w1t[k][:], in_=w_in[0][k * P:(k + 1) * P, :])
            w2t = const.tile([P, P], bf16, tag="w2")
            nc.sync.dma_start(out=w2t[:], in_=w_in[1][:, :])
            w3t = const.tile([DIMS[2], P], bf16, tag="w3")
            nc.sync.dma_start(out=w3t[:], in_=w_in[2][:, :])
            bt = []
            for i in range(3):
                t = const.tile([P, DIMS[i + 1]], f32, tag=f"b{i}", name=f"bt{i}")
                nc.sync.dma_start(out=t[:], in_=b_in[i][:, :])
                bt.append(t)
            def one_rep(rep):
                selft = [selfp.tile([P, NBLK, DIMS[i + 1]], bf16, tag=f"self{i}",
                                    name=f"self{i}_r{rep}") for i in range(3)]
                slabs = [dram.tile([SLABROWS, GD], bf16, tag=f"slab{i}",
                                   name=f"slab{i}_r{rep}") for i in range(3)]
                hbufs = [[dram.tile([CLSROWS, GD], bf16, tag=f"hbuf{i}_{q}",
                                    name=f"hbuf{i}_{q}_r{rep}", addr_space="Shared")
                          for q in range(NQ)] for i in range(3)]
                for l in range(3):
                    for q in range(NQ):
                        zr = q * CROWS + P * CLSBLK
                        nc.sync.dma_start(out=slabs[l][zr:zr + ZPAD, :], in_=zt[0:ZPAD, :])

            def allgather(layer, q):
                nc.gpsimd.collective_compute(
                    "AllGather", AL.bypass,
                    replica_groups=[list(range(NCORES))],
                    ins=[slabs[layer][q * CROWS:(q + 1) * CROWS, :].opt()],
                    outs=[hbufs[layer][q][:, :].opt()])

            def slab_view(layer, q):
                a = q * CROWS
                v = slabs[layer][a:a + P * CLSBLK, :]
                return v.rearrange("(p t) d -> p t d", p=P)

            def issue_gathers(layer):
                tiles = {}
                for cid, (q, blks, cols) in enumerate(calls):
                    gt = gpool.tile([P, cols, GD], bf16, tag="gt")
                    nidx = cols * P
                    nc.gpsimd.dma_gather(
                        out_ap=gt[:, :, :],
                        in_ap=hbufs[layer][q][MID:2 * MID, :],
                        idxs_ap=idx_t[:, int(cbase[cid]) // 16:(int(cbase[cid]) + nidx) // 16],
                        num_idxs=nidx, num_idxs_reg=nidx, elem_size=GD,
                        single_packet=False, queue_num=cid % nq_dma)
                    tiles[cid] = gt
                return tiles

            def reduce_block(tiles, j, dout, out_ap):
                # out_ap [P, dout] f32 = (sum of gathered) * dinv_j + self_j
                parts = []
                for q in range(NQ):
                    if blkq[j][q] is None:
                        continue
                    cid, off, kq = blkq[j][q]
                    view = tiles[cid][:, off:off + kq, :dout].rearrange("p k d -> p d k")
                    r = work.tile([P, dout], f32, tag=f"r{dout}")
                    nc.vector.tensor_reduce(out=r[:], in_=view,
                                            axis=mybir.AxisListType.X, op=AL.add)
                    parts.append(r)
                assert parts
                r = parts[0]
                if len(parts) == 2:
                    nc.vector.scalar_tensor_tensor(
                        out=r[:], in0=r[:], scalar=1.0, in1=parts[1][:],
                        op0=AL.mult, op1=AL.add)
                nc.vector.scalar_tensor_tensor(
                    out=out_ap, in0=r[:], scalar=dinv_t[:, j:j + 1],
                    in1=selft[0][:, j, :dout] if dout == DIMS[1]
                    else (selft[1][:, j, :dout] if dout == DIMS[2] else selft[2][:, j, :dout]),
                    op0=AL.mult, op1=AL.add)

            def transform_tail(i, j, b, src_ap, htb):
                # layer (i+2) transform of block j from pre-relu src [P, DIMS[i+1]]
                din = DIMS[i + 1]
                pt = ppt.tile([P, P], f32, space="PSUM", tag="trp")
                nc.tensor.transpose(out=pt[:din, :], in_=src_ap, identity=ident[:])
                sT = work.tile([din, P], bf16, tag=f"sT{din}")
                nc.scalar.activation(out=sT[:], in_=pt[:din, :],
                                     func=mybir.ActivationFunctionType.Relu)
                ps = pps.tile([P, P], f32, space="PSUM", tag="tps")
                wt = w2t if i == 0 else w3t
                nc.tensor.matmul(out=ps[:], lhsT=sT[:], rhs=wt[:], start=True, stop=True)
                nc.scalar.activation(out=htb[:, b, :], in_=ps[:],
                                     func=mybir.ActivationFunctionType.Copy,
                                     scale=dinv_t[:, j:j + 1])
                dnext = DIMS[i + 2]
                nc.vector.scalar_tensor_tensor(
                    out=selft[i + 1][:, j, :], in0=ps[:, :dnext],
                    scalar=dinv2_t[:, j:j + 1], in1=bt[i + 1][:],
                    op0=AL.mult, op1=AL.add)

            # ---- L1 transform: slab0 = dinv * (x @ W1); self0 = dinv^2*ps + b1
            xview = xT_in.rearrange("(k p) n -> p k n", p=P)
            for q in range(NQ):
                B0, B1 = q * CLSBLK, (q + 1) * CLSBLK
                for j0 in range(B0, B1, WB):
                    bsz = min(WB, B1 - j0)
                    lt = lhs.tile([P, 4, bsz * P], bf16, tag=f"xT{bsz}")
                    nc.sync.dma_start(out=lt[:], in_=xview[:, :, j0 * P:(j0 + bsz) * P])
                    htb = hbt.tile([P, bsz, GD], bf16, tag=f"htb{bsz}")
                    for b in range(bsz):
                        j = j0 + b
                        ps = pps.tile([P, P], f32, space="PSUM", tag="tps")
                        for k in range(4):
                            nc.tensor.matmul(out=ps[:], lhsT=lt[:, k, b * P:(b + 1) * P],
                                             rhs=w1t[k][:], start=(k == 0), stop=(k == 3))
                        nc.scalar.activation(out=htb[:, b, :], in_=ps[:],
                                             func=mybir.ActivationFunctionType.Copy,
                                             scale=dinv_t[:, j:j + 1])
                        nc.vector.scalar_tensor_tensor(
                            out=selft[0][:, j, :], in0=ps[:],
                            scalar=dinv2_t[:, j:j + 1], in1=bt[0][:],
                            op0=AL.mult, op1=AL.add)
                    nc.sync.dma_start(out=slab_view(0, q)[:, j0 - B0:j0 - B0 + bsz, :],
                                      in_=htb[:])
                allgather(0, q)

            # ---- agg layer i consumes hbufs[i], transforms into slabs[i+1]
            outv = out_t.rearrange("(p t) d -> p t d", p=P)
            for i in range(3):
                dout = DIMS[i + 1]
                tiles = issue_gathers(i)
                for q in range(NQ):
                    B0, B1 = q * CLSBLK, (q + 1) * CLSBLK
                    for j0 in range(B0, B1, WB):
                        bsz = min(WB, B1 - j0)
                        if i < 2:
                            htb = hbt.tile([P, bsz, GD], bf16, tag=f"htb{bsz}")
                        else:
                            htb = hbt.tile([P, bsz, DIMS[3]], f32, tag=f"ob{bsz}")
                        for b in range(bsz):
                            j = j0 + b
                            if i < 2:
                                s = work.tile([P, dout], f32, tag=f"s{dout}")
                                reduce_block(tiles, j, dout, s[:])
                                transform_tail(i, j, b, s[:], htb)
                            else:
                                reduce_block(tiles, j, dout, htb[:, b, :])
                        if i < 2:
                            nc.sync.dma_start(
                                out=slab_view(i + 1, q)[:, j0 - B0:j0 - B0 + bsz, :],
                                in_=htb[:])
                        else:
                            nc.sync.dma_start(out=outv[:, j0:j0 + bsz, :], in_=htb[:])
                    if i < 2:
                        allgather(i + 1, q)
    nc.compile()
    return nc


# --------------------------------------------------------------------------
# SPMD runner (shard_map over 8 axon cores, reusable jitted executable)
# --------------------------------------------------------------------------
class _Runner:
    def __init__(self, nc, n_cores=NCORES):
        import jax
        from jax.sharding import Mesh, PartitionSpec
        from jax.experimental.shard_map import shard_map
        from concourse import bass2jax, mybir
        bass2jax.install_neuronx_cc_hook()
        self.jax = jax
        self.n_cores = n_cores
        pname = nc.partition_id_tensor.name if nc.partition_id_tensor else None
        in_names, out_names, out_avals, zero_outs = [], [], [], []
        for alloc in nc.m.functions[0].allocations:
            if not isinstance(alloc, mybir.MemoryLocationSet):
                continue
            name = alloc.memorylocations[0].name
            if alloc.kind == "ExternalInput":
                if name != pname:
                    in_names.append(name)
            elif alloc.kind == "ExternalOutput":
                out_names.append(name)
                out_avals.append(jax.core.ShapedArray(tuple(alloc.tensor_shape), mybir.dt.np(alloc.dtype)))
                zero_outs.append(np.zeros(tuple(alloc.tensor_shape), mybir.dt.np(alloc.dtype)))
        self.in_names, self.out_names = in_names, out_names
        self.out_avals, self.zero_outs = out_avals, zero_outs
        n_params, n_outs = len(in_names), len(out_names)
        all_in = in_names + out_names + ([pname] if pname else [])

        def _body(*args):
            operands = list(args)
            if pname:
                operands.append(bass2jax.partition_id_tensor())
            outs = bass2jax._bass_exec_p.bind(
                *operands, out_avals=tuple(out_avals), in_names=tuple(all_in),
                out_names=tuple(out_names), lowering_input_output_aliases=(),
                sim_require_finite=True, sim_require_nnan=True, nc=nc)
            return tuple(outs)

        devices = jax.devices()[:n_cores]
        self.mesh = Mesh(np.asarray(devices), ("core",))
        self.pspec = PartitionSpec("core")
        self.fn = jax.jit(
            shard_map(_body, mesh=self.mesh,
                      in_specs=(self.pspec,) * (n_params + n_outs),
                      out_specs=(self.pspec,) * n_outs, check_rep=False),
            donate_argnums=tuple(range(n_params, n_params + n_outs)),
            keep_unused=True)

    def place(self, in_maps):
        sh = self.jax.sharding.NamedSharding(self.mesh, self.pspec)
        return [self.jax.device_put(
                    np.concatenate([np.asarray(in_maps[c][n]) for c in range(self.n_cores)], axis=0), sh)
                for n in self.in_names]

    def make_zeros(self):
        sh = self.jax.sharding.NamedSharding(self.mesh, self.pspec)
        zeros = [self.jax.device_put(
                    np.zeros((self.n_cores * z.shape[0], *z.shape[1:]), z.dtype), sh)
                 for z in self.zero_outs]
        self.jax.block_until_ready(zeros)
        return zeros

    def run(self, args, zeros=None):
        if zeros is None:
            zeros = self.make_zeros()
        outs = self.fn(*args, *zeros)
        self.jax.block_until_ready(outs)
        return outs

    def results(self, outs):
        return [{n: np.asarray(outs[i]).reshape(self.n_cores, *self.out_avals[i].shape)[c]
                 for i, n in enumerate(self.out_names)}
                for c in range(self.n_cores)]


# --------------------------------------------------------------------------
def _bf16(a):
    import ml_dtypes
    return np.asarray(a, dtype=ml_dtypes.bfloat16)


def build_in_maps(pre, x, W1, b1, W2, b2, W3, b3):
    ids = pre["ids"]                                  # [c, j, p] -> node
    x = np.asarray(x, np.float32)
    W2p = np.zeros((P, P), np.float32); W2p[:, :DIMS[2]] = np.asarray(W2, np.float32)
    W3p = np.zeros((DIMS[2], P), np.float32); W3p[:, :DIMS[3]] = np.asarray(W3, np.float32)
    in_maps = []
    for c in range(NCORES):
        v = ids[c].reshape(-1)                        # xT column j*128+p -> node
        xc = np.zeros((SLOTS, DIMS[0]), np.float32)
        m = v >= 0
        xc[m] = x[v[m]]
        in_maps.append({
            "xT": _bf16(np.ascontiguousarray(xc.T)),
            "gidx": pre["wrapped"][c],
            "dinv": pre["dinv_t"][c],
            "dinv2": pre["dinv2_t"][c],
            "W1": _bf16(np.asarray(W1, np.float32)),
            "W2": _bf16(W2p),
            "W3": _bf16(W3p),
            "b1": np.tile(np.asarray(b1, np.float32)[None, :], (P, 1)),
            "b2": np.tile(np.asarray(b2, np.float32)[None, :], (P, 1)),
            "b3": np.tile(np.asarray(b3, np.float32)[None, :], (P, 1)),
        })
    return in_maps


def _get(edge_index):
    key = hash(np.asarray(edge_index)[:, ::997].tobytes())
    if key not in _CACHE:
        pre = _preprocess(edge_index)
        nc = _build(pre)
        _CACHE[key] = (pre, _Runner(nc))
    return _CACHE[key]


def kernel(x, edge_index, W1, b1, W2, b2, W3, b3):
    pre, runner = _get(edge_index)
    in_maps = build_in_maps(pre, x, W1, b1, W2, b2, W3, b3)
    args = runner.place(in_maps)
    outs = runner.run(args)
    res = runner.results(outs)
    ids = pre["ids"]
    full = np.zeros((N, DIMS[3]), np.float32)
    for c in range(NCORES):
        v = ids[c]                                    # [j, p]
        # out row = p*NBLK + j (lane-major)
        o = res[c]["out"].reshape(P, NBLK, DIMS[3])
        m = v >= 0
        jj, pp = np.nonzero(m)
        full[v[jj, pp]] = o[pp, jj]
    return full


# revision 26
# speedup vs baseline: 1.6742x; 1.0999x over previous
"""3-layer GCN (PyG GCNConv x3, relu between) on 8 Trainium2 NeuronCores.

Math: out = A*(relu(A*(relu(A*(xW1)+b1)W2+b2))W3)+b3 with A = D^-1/2(A+I)D^-1/2.
Per layer: htilde = dinv * (input @ W) (dense, PE, bf16) is written to a
per-core slab and AllGathered per class (2 tables of 50,304 rows; class-0's
AG fires at the producer loop's midpoint, overlapping its second half);
aggregation of the REAL edges is a dma_gather + per-block strided
tensor_reduce; the self-loop term dinv^2*(input@W)[dst] + b is kept in SBUF
from the transform phase and added via one scalar_tensor_tensor, so
self-loops never transit the gather path.

Gather indexing: dma_gather takes int16 row indices. Each call's in_ap is
based at the middle of its class table, idx = row - MID in [-25152, 25151]
(HW sign-extends; only a negative index in the *final* list position is
dropped, so pads are positive and each call's last entry is forced
non-negative by an in-lane swap). Host preprocessing greedily 2-colors SRC
nodes (symmetric balance objective + 6 refinement sweeps) to split every
dst's in-edges evenly across the classes, then packs dsts with similar
(deg, class-0 count) into the same 128-lane block (snake order over (d, c0)
so block boundaries don't mix low-c0 and high-c0 extremes). Consecutive
blocks of one class share a single gather call (up to 4096 idxs) to
amortize the ~1us SWDGE fixed cost; class-1 calls are staggered after
class-0 ones so the in-order gpsimd queue isn't blocked by the class-1 AG.
Pad entries cycle over 8*ZPAD distinct zero rows -- descriptors that all
hit one row serialize at the HBM bank (~13.5 ns each, measured).

Layout: dst slot (core, block j, lane p). xT columns are block-major
(j*128+p); table/slab rows are lane-major within each class
(p*CLSBLK + j - q*CLSBLK) so 8-block batches of transform output form one
contiguous-per-partition DMA; out rows are lane-major global (p*98+j).
Each (core, class) appends ZPAD zero slots (gather pad targets).
"""
import sys
sys.path.insert(0, "/opt/trn_rl_repo")
import numpy as np

N = 100_000
DIMS = [512, 128, 64, 32]
NCORES = 8
P = 128
NBLK = 98
GD = 128
SLOTS = NBLK * P                 # 12544 xT columns / out rows per core
NQ = 2
CLSBLK = NBLK // NQ              # 49 blocks per class
ZPAD = 16                        # zero slots per (core, class): pad targets
                                 # spread over 8*ZPAD rows -- same-row gathers
                                 # serialize at the HBM bank (~13.5 ns/desc)
CROWS = CLSBLK * P + ZPAD        # per-core rows per class
SLABROWS = NQ * CROWS            # 12576
CLSROWS = NCORES * CROWS         # 50304 rows per class table
MID = CLSROWS // 2               # 25152
CALL_CAP_COLS = 32               # 4096 idxs per gather call
WB = 8                           # transform write batch (blocks)

_CACHE = {}


# --------------------------------------------------------------------------
# host-side graph preprocessing
# --------------------------------------------------------------------------
def _preprocess(edge_index):
    src = np.asarray(edge_index[0], np.int64)
    dst = np.asarray(edge_index[1], np.int64)
    indeg = np.bincount(dst, minlength=N).astype(np.int64)
    deg = indeg + 1                                   # + self loop (norm)
    dinv = (1.0 / np.sqrt(deg)).astype(np.float32)

    # ---- 2-coloring of SRC nodes: balance each dst's in-edges across halves
    order = np.argsort(src, kind="stable")
    S_srt, D_srt = src[order], dst[order]
    indptr = np.searchsorted(S_srt, np.arange(N + 1))
    M = np.zeros((N, NQ), np.int32)
    cls = np.full(N, -1, np.int8)
    capq = np.full(NQ, 49 * 1024, np.int64)
    proc = np.argsort(-(indptr[1:] - indptr[:-1]), kind="stable")
    for n in proc:
        nb = D_srt[indptr[n]:indptr[n + 1]]
        diff = int((M[nb, 0] - M[nb, 1]).sum())
        q = 0 if diff < 0 else 1
        if capq[q] <= 0:
            q = 1 - q
        cls[n] = q
        capq[q] -= 1
        M[nb, q] += 1
    for _ in range(6):
        moved = 0
        for n in proc:
            nb = D_srt[indptr[n]:indptr[n + 1]]
            q0 = cls[n]
            M[nb, q0] -= 1
            capq[q0] += 1
            diff = int((M[nb, 0] - M[nb, 1]).sum())
            q = 0 if diff < 0 else 1
            if capq[q] <= 0:
                q = 1 - q
            moved += q != q0
            cls[n] = q
            capq[q] -= 1
            M[nb, q] += 1
        if moved == 0:
            break

    # ---- pack dsts into blocks (snake order on (deg, c0)); class q -> its
    # 49 blocks; within a block, position t -> core t//128, lane t%128
    core_of = np.full(N, -1, np.int64)
    blk_of = np.full(N, -1, np.int64)
    lane_of = np.full(N, -1, np.int64)
    for q in range(NQ):
        nodes_q = np.flatnonzero(cls == q)
        c0 = M[nodes_q, 0].astype(np.int64)
        d = c0 + M[nodes_q, 1]
        key2 = np.where(d % 2 == 0, c0, 100000 - c0)
        stream = nodes_q[np.lexsort((key2, d))]
        jbase = q * CLSBLK
        for b in range(CLSBLK):
            seg = stream[b * 1024:(b + 1) * 1024]
            t = np.arange(len(seg))
            core_of[seg] = t // P
            blk_of[seg] = jbase + b
            lane_of[seg] = t % P

    # ---- table rows: class tensor row = c*CROWS + p*CLSBLK + (j - q*CLSBLK)
    ec, ej, el = core_of[dst], blk_of[dst], lane_of[dst]
    eq = cls[src].astype(np.int64)
    srow = (core_of[src] * CROWS + lane_of[src] * CLSBLK
            + (blk_of[src] - eq * CLSBLK))
    eidx = srow - MID
    assert eidx.min() >= -MID and eidx.max() < MID

    # per-(core, class, block, lane) counts -> K[q, j]
    key = ((ec * NQ + eq) * NBLK + ej) * P + el
    cnt = np.zeros(NCORES * NQ * NBLK * P, np.int64)
    np.add.at(cnt, key, 1)
    cnt4 = cnt.reshape(NCORES, NQ, NBLK, P)
    K = cnt4.max(axis=(0, 3))                         # [NQ, NBLK]

    order_e = np.argsort(key, kind="stable")
    ks = key[order_e]
    first = np.flatnonzero(np.r_[True, ks[1:] != ks[:-1]])
    within = np.arange(len(ks)) - first[np.searchsorted(ks[first], ks)]
    oc, oq, oj, ol = ec[order_e], eq[order_e], ej[order_e], el[order_e]
    oidx = eidx[order_e]

    # pad targets: cycle over the upper-half zero rows (cores 4..7, all
    # positive idx so any of them can legally terminate a call)
    pad_pool = np.array([c * CROWS + CLSBLK * P + t - MID
                         for c in range(4, 8) for t in range(ZPAD)], np.int64)
    assert (pad_pool >= 0).all() and pad_pool.max() < MID

    def build(K):
        # calls: per class, consecutive blocks, <= CALL_CAP_COLS columns
        calls = []                                    # (q, [(j, off, K)], cols)
        for q in range(NQ):
            cur, cols = [], 0
            for j in range(NBLK):
                kqj = int(K[q, j])
                if kqj == 0:
                    continue
                if cols + kqj > CALL_CAP_COLS and cur:
                    calls.append((q, cur, cols))
                    cur, cols = [], 0
                cur.append((j, cols, kqj))
                cols += kqj
            if cur:
                calls.append((q, cur, cols))
        # class-1 calls are staggered ~10 blocks later in issue order: the
        # gpsimd queue is in-order, and the first class-1 call blocks on the
        # class-1 AllGather mesh; meanwhile class-0 calls (whose AG completed
        # mid-producer-loop) keep the DMA engines fed.
        calls.sort(key=lambda cc: (cc[1][0][0] + (10 if cc[0] == 1 else 0), cc[0]))
        cbase = np.zeros(len(calls), np.int64)
        acc = 0
        cidm = np.full((NQ, NBLK), -1, np.int64)
        offm = np.zeros((NQ, NBLK), np.int64)
        blkq = [[None] * NQ for _ in range(NBLK)]
        for cid, (q, blks, cols) in enumerate(calls):
            cbase[cid] = acc
            acc += cols * P
            for (j, off, kqj) in blks:
                cidm[q, j], offm[q, j] = cid, off
                blkq[j][q] = (cid, off, kqj)
        total_idx = acc

        req = np.empty((NCORES, total_idx), np.int64)
        for c in range(NCORES):
            req[c] = pad_pool[(np.arange(total_idx) + 17 * c) % len(pad_pool)]
        ecall = cidm[oq, oj]
        ekoff = offm[oq, oj] + within
        req[oc, cbase[ecall] + ekoff * P + ol] = oidx
        return calls, cbase, blkq, total_idx, req

    for _ in range(10):
        calls, cbase, blkq, total_idx, req = build(K)
        bumped = False
        for cid, (q, blks, cols) in enumerate(calls):
            e = cbase[cid] + cols * P - 1
            jl, offl, kl = blks[-1]
            # final entry = lane 127, top-k of the call's last block; if
            # negative, swap with a non-negative entry in the same lane/block
            # k-range (order within a k-range is irrelevant to the sum)
            cols_l = cbase[cid] + (offl + np.arange(kl)) * P + (P - 1)
            for c in range(NCORES):
                if req[c, e] < 0:
                    cand = np.flatnonzero(req[c, cols_l] >= 0)
                    if len(cand) == 0:
                        K[q, jl] += 1
                        bumped = True
                        break
                    s_ = cols_l[cand[0]]
                    req[c, e], req[c, s_] = req[c, s_], req[c, e]
            if bumped:
                break
        if not bumped:
            break
    assert req.max() < 32768 and req.min() >= -32768

    wrapped = np.empty((NCORES, P, total_idx // 16), np.int16)
    for c in range(NCORES):
        col = 0
        for cid, (q, blks, cols) in enumerate(calls):
            L = req[c, cbase[cid]:cbase[cid] + cols * P]
            w = L.reshape(-1, 16).T.astype(np.int16)
            wrapped[c, :, col:col + cols * P // 16] = np.tile(w, (8, 1))
            col += cols * P // 16

    # ---- per-core node placement tables
    ids = np.full((NCORES, NBLK, P), -1, np.int64)    # [c, j, p] -> node
    m = core_of >= 0
    ids[core_of[m], blk_of[m], lane_of[m]] = np.flatnonzero(m)
    dinv_t = np.zeros((NCORES, P, NBLK), np.float32)
    dinv2_t = np.zeros((NCORES, P, NBLK), np.float32)
    for c in range(NCORES):
        v = ids[c]                                    # [j, p]
        dv = np.where(v >= 0, dinv[np.maximum(v, 0)], 0.0).astype(np.float32)
        dinv_t[c] = dv.T
        dinv2_t[c] = (dv * dv).T
    return dict(ids=ids, calls=calls, cbase=cbase, blkq=blkq,
                wrapped=wrapped, dinv_t=dinv_t, dinv2_t=dinv2_t,
                total_idx=total_idx)


# --------------------------------------------------------------------------
# bass program
# --------------------------------------------------------------------------
def _build(pre, reps=1, nq_dma=4):
    from concourse import bass, bacc, mybir, tile
    from concourse.library_config import mlp
    from concourse.masks import make_identity
    AL = mybir.AluOpType
    f32, bf16, i16 = mybir.dt.float32, mybir.dt.bfloat16, mybir.dt.int16
    calls, blkq, cbase = pre["calls"], pre["blkq"], pre["cbase"]
    total_idx = pre["total_idx"]

    nc = bacc.Bacc("TRN2", target_bir_lowering=False, debug=False,
                   num_devices=NCORES, num_swdge_queues=nq_dma)
    xT_in = nc.dram_tensor("xT", (DIMS[0], SLOTS), bf16, kind="ExternalInput")
    idx_in = nc.dram_tensor("gidx", (P, total_idx // 16), i16, kind="ExternalInput")
    dinv_in = nc.dram_tensor("dinv", (P, NBLK), f32, kind="ExternalInput")
    dinv2_in = nc.dram_tensor("dinv2", (P, NBLK), f32, kind="ExternalInput")
    w_in = [nc.dram_tensor("W1", (DIMS[0], P), bf16, kind="ExternalInput"),
            nc.dram_tensor("W2", (P, P), bf16, kind="ExternalInput"),
            nc.dram_tensor("W3", (DIMS[2], P), bf16, kind="ExternalInput")]
    b_in = [nc.dram_tensor(f"b{i+1}", (P, DIMS[i + 1]), f32, kind="ExternalInput")
            for i in range(3)]
    out_t = nc.dram_tensor("out", (SLOTS, DIMS[3]), f32, kind="ExternalOutput")

    with tile.TileContext(nc) as tc:
        with tc.tile_pool(name="const", bufs=1) as const, \
             tc.tile_pool(name="selfp", bufs=1) as selfp, \
             tc.tile_pool(name="gath", bufs=10) as gpool, \
             tc.tile_pool(name="work", bufs=4) as work, \
             tc.tile_pool(name="hbt", bufs=3) as hbt, \
             tc.tile_pool(name="lhs", bufs=2) as lhs, \
             tc.tile_pool(name="pps", bufs=2, space="PSUM") as pps, \
             tc.tile_pool(name="ppt", bufs=2, space="PSUM") as ppt, \
             tc.tile_pool(name="dram", bufs=1, space="DRAM") as dram:

            nc.gpsimd.load_library(mlp)
            idx_t = const.tile([P, total_idx // 16], i16, tag="idx")
            nc.sync.dma_start(out=idx_t[:], in_=idx_in[:, :])
            dinv_t = const.tile([P, NBLK], f32, tag="dinv")
            nc.sync.dma_start(out=dinv_t[:], in_=dinv_in[:, :])
            dinv2_t = const.tile([P, NBLK], f32, tag="dinv2")
            nc.sync.dma_start(out=dinv2_t[:], in_=dinv2_in[:, :])
            ident = const.tile([P, P], f32, tag="ident")
            make_identity(nc, ident[:])
            zt = const.tile([P, GD], bf16, tag="zero")
            nc.vector.memset(zt[:], 0.0)
            w1t = [const.tile([P, P], bf16, tag=f"w1_{k}", name=f"w1_{k}") for k in range(4)]
            for k in range(4):
                nc.sync.dma_start(out=w1t[k][:], in_=w_in[0][k * P:(k + 1) * P, :])
            w2t = const.tile([P, P], bf16, tag="w2")
            nc.sync.dma_start(out=w2t[:], in_=w_in[1][:, :])
            w3t = const.tile([DIMS[2], P], bf16, tag="w3")
            nc.sync.dma_start(out=w3t[:], in_=w_in[2][:, :])
            bt = []
            for i in range(3):
                t = const.tile([P, DIMS[i + 1]], f32, tag=f"b{i}", name=f"bt{i}")
                nc.sync.dma_start(out=t[:], in_=b_in[i][:, :])
                bt.append(t)
            def one_rep(rep):
                selft = [selfp.tile([P, NBLK, DIMS[i + 1]], bf16, tag=f"self{i}",
                                    name=f"self{i}_r{rep}") for i in range(3)]
                slabs = [dram.tile([SLABROWS, GD], bf16, tag=f"slab{i}",
                                   name=f"slab{i}_r{rep}") for i in range(3)]
                hbufs = [[dram.tile([CLSROWS, GD], bf16, tag=f"hbuf{i}_{q}",
                                    name=f"hbuf{i}_{q}_r{rep}", addr_space="Shared")
                          for q in range(NQ)] for i in range(3)]
                for l in range(3):
                    for q in range(NQ):
                        zr = q * CROWS + P * CLSBLK
                        nc.sync.dma_start(out=slabs[l][zr:zr + ZPAD, :], in_=zt[0:ZPAD, :])

            def allgather(layer, q):
                nc.gpsimd.collective_compute(
                    "AllGather", AL.bypass,
                    replica_groups=[list(range(NCORES))],
                    ins=[slabs[layer][q * CROWS:(q + 1) * CROWS, :].opt()],
                    outs=[hbufs[layer][q][:, :].opt()])

            def slab_view(layer, q):
                a = q * CROWS
                v = slabs[layer][a:a + P * CLSBLK, :]
                return v.rearrange("(p t) d -> p t d", p=P)

            def issue_gathers(layer):
                tiles = {}
                for cid, (q, blks, cols) in enumerate(calls):
                    gt = gpool.tile([P, cols, GD], bf16, tag="gt")
                    nidx = cols * P
                    nc.gpsimd.dma_gather(
                        out_ap=gt[:, :, :],
                        in_ap=hbufs[layer][q][MID:2 * MID, :],
                        idxs_ap=idx_t[:, int(cbase[cid]) // 16:(int(cbase[cid]) + nidx) // 16],
                        num_idxs=nidx, num_idxs_reg=nidx, elem_size=GD,
                        single_packet=False, queue_num=cid % nq_dma)
                    tiles[cid] = gt
                return tiles

            def reduce_block(tiles, j, dout, out_ap):
                # out_ap [P, dout] f32 = (sum of gathered) * dinv_j + self_j
                parts = []
                for q in range(NQ):
                    if blkq[j][q] is None:
                        continue
                    cid, off, kq = blkq[j][q]
                    view = tiles[cid][:, off:off + kq, :dout].rearrange("p k d -> p d k")
                    r = work.tile([P, dout], f32, tag=f"r{dout}")
                    nc.vector.tensor_reduce(out=r[:], in_=view,
                                            axis=mybir.AxisListType.X, op=AL.add)
                    parts.append(r)
                assert parts
                r = parts[0]
                if len(parts) == 2:
                    nc.vector.scalar_tensor_tensor(
                        out=r[:], in0=r[:], scalar=1.0, in1=parts[1][:],
                        op0=AL.mult, op1=AL.add)
                nc.vector.scalar_tensor_tensor(
                    out=out_ap, in0=r[:], scalar=dinv_t[:, j:j + 1],
                    in1=selft[0][:, j, :dout] if dout == DIMS[1]
                    else (selft[1][:, j, :dout] if dout == DIMS[2] else selft[2][:, j, :dout]),
                    op0=AL.mult, op1=AL.add)

            def transform_tail(i, j, b, src_ap, htb):
                # layer (i+2) transform of block j from pre-relu src [P, DIMS[i+1]]
                din = DIMS[i + 1]
                pt = ppt.tile([P, P], f32, space="PSUM", tag="trp")
                nc.tensor.transpose(out=pt[:din, :], in_=src_ap, identity=ident[:])
                sT = work.tile([din, P], bf16, tag=f"sT{din}")
                nc.scalar.activation(out=sT[:], in_=pt[:din, :],
                                     func=mybir.ActivationFunctionType.Relu)
                ps = pps.tile([P, P], f32, space="PSUM", tag="tps")
                wt = w2t if i == 0 else w3t
                nc.tensor.matmul(out=ps[:], lhsT=sT[:], rhs=wt[:], start=True, stop=True)
                nc.scalar.activation(out=htb[:, b, :], in_=ps[:],
                                     func=mybir.ActivationFunctionType.Copy,
                                     scale=dinv_t[:, j:j + 1])
                dnext = DIMS[i + 2]
                nc.vector.scalar_tensor_tensor(
                    out=selft[i + 1][:, j, :], in0=ps[:, :dnext],
                    scalar=dinv2_t[:, j:j + 1], in1=bt[i + 1][:],
                    op0=AL.mult, op1=AL.add)

            # ---- L1 transform: slab0 = dinv * (x @ W1); self0 = dinv^2*ps + b1
            xview = xT_in.rearrange("(k p) n -> p k n", p=P)
            for q in range(NQ):
                B0, B1 = q * CLSBLK, (q + 1) * CLSBLK
                for j0 in range(B0, B1, WB):
                    bsz = min(WB, B1 - j0)
                    lt = lhs.tile([P, 4, bsz * P], bf16, tag=f"xT{bsz}")
                    nc.sync.dma_start(out=lt[:], in_=xview[:, :, j0 * P:(j0 + bsz) * P])
                    htb = hbt.tile([P, bsz, GD], bf16, tag=f"htb{bsz}")
                    for b in range(bsz):
                        j = j0 + b
                        ps = pps.tile([P, P], f32, space="PSUM", tag="tps")
                        for k in range(4):
                            nc.tensor.matmul(out=ps[:], lhsT=lt[:, k, b * P:(b + 1) * P],
                                             rhs=w1t[k][:], start=(k == 0), stop=(k == 3))
                        nc.scalar.activation(out=htb[:, b, :], in_=ps[:],
                                             func=mybir.ActivationFunctionType.Copy,
                                             scale=dinv_t[:, j:j + 1])
                        nc.vector.scalar_tensor_tensor(
                            out=selft[0][:, j, :], in0=ps[:],
                            scalar=dinv2_t[:, j:j + 1], in1=bt[0][:],
                            op0=AL.mult, op1=AL.add)
                    nc.sync.dma_start(out=slab_view(0, q)[:, j0 - B0:j0 - B0 + bsz, :],
                                      in_=htb[:])
                allgather(0, q)

            # ---- agg layer i consumes hbufs[i], transforms into slabs[i+1]
            outv = out_t.rearrange("(p t) d -> p t d", p=P)
            for i in range(3):
                dout = DIMS[i + 1]
                tiles = issue_gathers(i)
                for q in range(NQ):
                    B0, B1 = q * CLSBLK, (q + 1) * CLSBLK
                    for j0 in range(B0, B1, WB):
                        bsz = min(WB, B1 - j0)
                        if i < 2:
                            htb = hbt.tile([P, bsz, GD], bf16, tag=f"htb{bsz}")
                        else:
                            htb = hbt.tile([P, bsz, DIMS[3]], f32, tag=f"ob{bsz}")
                        for b in range(bsz):
                            j = j0 + b
                            if i < 2:
                                s = work.tile([P, dout], f32, tag=f"s{dout}")
                                reduce_block(tiles, j, dout, s[:])
                                transform_tail(i, j, b, s[:], htb)
                            else:
                                reduce_block(tiles, j, dout, htb[:, b, :])
                        if i < 2:
                            nc.sync.dma_start(
                                out=slab_view(i + 1, q)[:, j0 - B0:j0 - B0 + bsz, :],
                                in_=htb[:])
                        else:
                            nc.sync.dma_start(out=outv[:, j0:j0 + bsz, :], in_=htb[:])
                    if i < 2:
                        allgather(i + 1, q)
    nc.compile()
    return nc


# --------------------------------------------------------------------------
# SPMD runner (shard_map over 8 axon cores, reusable jitted executable)
# --------------------------------------------------------------------------
class _Runner:
    def __init__(self, nc, n_cores=NCORES):
        import jax
        from jax.sharding import Mesh, PartitionSpec
        from jax.experimental.shard_map import shard_map
        from concourse import bass2jax, mybir
        bass2jax.install_neuronx_cc_hook()
        self.jax = jax
        self.n_cores = n_cores
        pname = nc.partition_id_tensor.name if nc.partition_id_tensor else None
        in_names, out_names, out_avals, zero_outs = [], [], [], []
        for alloc in nc.m.functions[0].allocations:
            if not isinstance(alloc, mybir.MemoryLocationSet):
                continue
            name = alloc.memorylocations[0].name
            if alloc.kind == "ExternalInput":
                if name != pname:
                    in_names.append(name)
            elif alloc.kind == "ExternalOutput":
                out_names.append(name)
                out_avals.append(jax.core.ShapedArray(tuple(alloc.tensor_shape), mybir.dt.np(alloc.dtype)))
                zero_outs.append(np.zeros(tuple(alloc.tensor_shape), mybir.dt.np(alloc.dtype)))
        self.in_names, self.out_names = in_names, out_names
        self.out_avals, self.zero_outs = out_avals, zero_outs
        n_params, n_outs = len(in_names), len(out_names)
        all_in = in_names + out_names + ([pname] if pname else [])

        def _body(*args):
            operands = list(args)
            if pname:
                operands.append(bass2jax.partition_id_tensor())
            outs = bass2jax._bass_exec_p.bind(
                *operands, out_avals=tuple(out_avals), in_names=tuple(all_in),
                out_names=tuple(out_names), lowering_input_output_aliases=(),
                sim_require_finite=True, sim_require_nnan=True, nc=nc)
            return tuple(outs)

        devices = jax.devices()[:n_cores]
        self.mesh = Mesh(np.asarray(devices), ("core",))
        self.pspec = PartitionSpec("core")
        self.fn = jax.jit(
            shard_map(_body, mesh=self.mesh,
                      in_specs=(self.pspec,) * (n_params + n_outs),
                      out_specs=(self.pspec,) * n_outs, check_rep=False),
            donate_argnums=tuple(range(n_params, n_params + n_outs)),
            keep_unused=True)

    def place(self, in_maps):
        sh = self.jax.sharding.NamedSharding(self.mesh, self.pspec)
        return [self.jax.device_put(
                    np.concatenate([np.asarray(in_maps[c][n]) for c in range(self.n_cores)], axis=0), sh)
                for n in self.in_names]

    def make_zeros(self):
        sh = self.jax.sharding.NamedSharding(self.mesh, self.pspec)
        zeros = [self.jax.device_put(
                    np.zeros((self.n_cores * z.shape[0], *z.shape[1:]), z.dtype), sh)
                 for z in self.zero_outs]
        self.jax.block_until_ready(zeros)
        return zeros

    def run(self, args, zeros=None):
        if zeros is None:
            zeros = self.make_zeros()
        outs = self.fn(*args, *zeros)
        self.jax.block_until_ready(outs)
        return outs

    def results(self, outs):
        return [{n: np.asarray(outs[i]).reshape(self.n_cores, *self.out_avals[i].shape)[c]
                 for i, n in enumerate(self.out_names)}
                for c in range(self.n_cores)]


# --------------------------------------------------------------------------
def _bf16(a):
    import ml_dtypes
    return np.asarray(a, dtype=ml_dtypes.bfloat16)


def build_in_maps(pre, x, W1, b1, W2, b2, W3, b3):
    ids = pre["ids"]                                  # [c, j, p] -> node
    x = np.asarray(x, np.float32)
    W2p = np.zeros((P, P), np.float32); W2p[:, :DIMS[2]] = np.asarray(W2, np.float32)
    W3p = np.zeros((DIMS[2], P), np.float32); W3p[:, :DIMS[3]] = np.asarray(W3, np.float32)
    in_maps = []
    for c in range(NCORES):
        v = ids[c].reshape(-1)                        # xT column j*128+p -> node
        xc = np.zeros((SLOTS, DIMS[0]), np.float32)
        m = v >= 0
        xc[m] = x[v[m]]
        in_maps.append({
            "xT": _bf16(np.ascontiguousarray(xc.T)),
            "gidx": pre["wrapped"][c],
            "dinv": pre["dinv_t"][c],
            "dinv2": pre["dinv2_t"][c],
            "W1": _bf16(np.asarray(W1, np.float32)),
            "W2": _bf16(W2p),
            "W3": _bf16(W3p),
            "b1": np.tile(np.asarray(b1, np.float32)[None, :], (P, 1)),
            "b2": np.tile(np.asarray(b2, np.float32)[None, :], (P, 1)),
            "b3": np.tile(np.asarray(b3, np.float32)[None, :], (P, 1)),
        })
    return in_maps


def _get(edge_index):
    key = hash(np.asarray(edge_index)[:, ::997].tobytes())
    if key not in _CACHE:
        pre = _preprocess(edge_index)
        nc = _build(pre)
        _CACHE[key] = (pre, _Runner(nc))
    return _CACHE[key]


def kernel(x, edge_index, W1, b1, W2, b2, W3, b3):
    pre, runner = _get(edge_index)
    in_maps = build_in_maps(pre, x, W1, b1, W2, b2, W3, b3)
    args = runner.place(in_maps)
    outs = runner.run(args)
    res = runner.results(outs)
    ids = pre["ids"]
    full = np.zeros((N, DIMS[3]), np.float32)
    for c in range(NCORES):
        v = ids[c]                                    # [j, p]
        # out row = p*NBLK + j (lane-major)
        o = res[c]["out"].reshape(P, NBLK, DIMS[3])
        m = v >= 0
        jj, pp = np.nonzero(m)
        full[v[jj, pp]] = o[pp, jj]
    return full
